# revision 26
# baseline (speedup 1.0000x reference)
"""GQA decode attention (B=32, S=1, 32 Q heads / 8 KV heads, HD=128, T=4096)
for 8 Trainium2 NeuronCores, tensor-parallel over heads.

Per core g: 4 query heads (4g..4g+3) + KV head g.
  - QKV projections from x (contraction over DIM=4096 on PE, fp32r)
  - RoPE on q/k (DVE, strided even/odd APs), 1/sqrt(HD) folded into q's rope
  - scores[bh, t] via zero-padded per-batch q weights (one PSUM bank per
    512-wide T chunk, accumulated over the 32 batches)
  - new-token k patched into the streamed K^T tile column 4095
  - softmax over the free axis on full 128 partitions (b*4+h), exp on ACT
    with fused accumulated row-sums; p normalized in place
  - p transposed per 128-chunk on PE; PV with V as stationary operand,
    output [d, bh] accumulated in one PSUM bank; new-token v added via one
    masked rank-32 matmul correction
  - out projection wo with attnT column-strided weights

Host side: shards weights/caches per head, pre-transposes K cache to
[B, HD, T] and pre-swizzles V cache to [B, 128, 32, HD] so every DMA is
16KB-contiguous per partition. Partial outputs (one per core) summed on host.
"""

import numpy as np

B, DIM, NH, NKV, HD = 32, 4096, 32, 8, 128
T = 4096
NCORES = 8
HPC = NH // NCORES            # 4 query heads per core
OUTW = HPC * HD               # 512
ALPHA = float(1.0 / np.sqrt(HD))
DC = DIM // 128               # 32 contraction chunks for projections
TC = T // 512                 # 8 score chunks (512 wide)
PC = T // 128                 # 32 PV chunks (128 deep)


def build_nc(dbg=False, corr=True):
    import concourse.mybir as mybir
    import concourse.tile as tile
    from concourse import bacc

    f32 = mybir.dt.float32
    f32r = mybir.dt.float32r
    X = mybir.AxisListType.X
    EXP = mybir.ActivationFunctionType.Exp
    SUB = mybir.AluOpType.subtract

    nc = bacc.Bacc("TRN2", target_bir_lowering=False, debug=False,
                   num_devices=NCORES)

    xT = nc.dram_tensor("xT", [DIM, B], f32, kind="ExternalInput")
    wq = nc.dram_tensor("wq", [DIM, OUTW], f32, kind="ExternalInput")
    wk = nc.dram_tensor("wk", [DIM, HD], f32, kind="ExternalInput")
    wv = nc.dram_tensor("wv", [DIM, HD], f32, kind="ExternalInput")
    wo = nc.dram_tensor("wo", [OUTW, DIM], f32, kind="ExternalInput")
    kt = nc.dram_tensor("kt", [B, HD, T], f32, kind="ExternalInput")
    vc = nc.dram_tensor("vc", [B, 128, PC, HD], f32, kind="ExternalInput")
    csq = nc.dram_tensor("csq", [2, OUTW // 2], f32, kind="ExternalInput")
    csk = nc.dram_tensor("csk", [2, HD // 2], f32, kind="ExternalInput")
    ones = nc.dram_tensor("ones", [1, 128], f32, kind="ExternalInput")
    iden = nc.dram_tensor("iden", [128, 128], f32, kind="ExternalInput")
    outp = nc.dram_tensor("outp", [B, DIM], f32, kind="ExternalOutput")
    if dbg:
        dbg_qrot = nc.dram_tensor("dbg_qrot", [B, OUTW], f32,
                                  kind="ExternalOutput")
        dbg_krot = nc.dram_tensor("dbg_krot", [B, HD], f32,
                                  kind="ExternalOutput")
        dbg_scores = nc.dram_tensor("dbg_scores", [128, T], f32,
                                    kind="ExternalOutput")
        dbg_p = nc.dram_tensor("dbg_p", [128, T], f32, kind="ExternalOutput")
        dbg_attnT = nc.dram_tensor("dbg_attnT", [128, B * HPC], f32,
                                   kind="ExternalOutput")
        dbg_prow = nc.dram_tensor("dbg_prow", [1, 128], f32,
                                  kind="ExternalOutput")

    def r(ap):
        return ap.bitcast(f32r)

    with tile.TileContext(nc) as tc:
        with (
            tc.tile_pool(name="pp", bufs=1) as pp,
            tc.tile_pool(name="wp", bufs=6) as wp,
            tc.tile_pool(name="wkp", bufs=3) as wkp,
            tc.tile_pool(name="ktp", bufs=2) as ktp,
            tc.tile_pool(name="vp", bufs=3) as vp,
            tc.tile_pool(name="qxp", bufs=3) as qxp,
        ):
            # ------- constants & persistent tiles
            xT_sb = pp.tile([128, DC, B], f32, tag="xT_sb")
            nc.sync.dma_start(r(xT_sb),
                              r(xT[:].rearrange("(dc p) b -> p dc b", p=128)))
            iden_sb = pp.tile([128, 128], f32, tag="iden_sb")
            nc.sync.dma_start(iden_sb, iden[:])
            ones_sb = pp.tile([1, 128], f32, tag="ones_sb")
            nc.sync.dma_start(r(ones_sb), r(ones[:]))
            cq32 = pp.tile([B, OUTW // 2], f32, tag="cq32")
            nc.sync.dma_start(cq32, csq[0:1, :].to_broadcast([B, OUTW // 2]))
            sq32 = pp.tile([B, OUTW // 2], f32, tag="sq32")
            nc.sync.dma_start(sq32, csq[1:2, :].to_broadcast([B, OUTW // 2]))
            ck32 = pp.tile([B, HD // 2], f32, tag="ck32")
            nc.sync.dma_start(ck32, csk[0:1, :].to_broadcast([B, HD // 2]))
            sk32 = pp.tile([B, HD // 2], f32, tag="sk32")
            nc.sync.dma_start(sk32, csk[1:2, :].to_broadcast([B, HD // 2]))
            zero1 = pp.tile([128, 1], f32, tag="zero1")
            nc.vector.memset(zero1, 0.0)

            # ------- phase A: projections + rope + q/k transposes
            with tc.tile_pool(name="psA", bufs=1, space="PSUM") as psA:
                psq = psA.tile([B, OUTW], f32, tag="psq")
                for dc in range(DC):
                    wqt = wp.tile([128, OUTW], f32, tag="wchunk",
                                  name=f"wqt{dc}")
                    nc.sync.dma_start(
                        r(wqt),
                        r(wq[:].rearrange("(dc p) o -> dc p o", p=128)[dc]))
                    nc.tensor.matmul(psq, r(xT_sb[:, dc, :]), r(wqt),
                                     start=(dc == 0), stop=(dc == DC - 1))
                psk = psA.tile([B, HD], f32, tag="psk")
                for dc in range(DC):
                    wkt = wkp.tile([128, HD], f32, tag="wkt",
                                   name=f"wkt{dc}")
                    nc.sync.dma_start(
                        r(wkt),
                        r(wk[:].rearrange("(dc p) o -> dc p o", p=128)[dc]))
                    nc.tensor.matmul(psk, r(xT_sb[:, dc, :]), r(wkt),
                                     start=(dc == 0), stop=(dc == DC - 1))
                psv = psA.tile([B, HD], f32, tag="psv")
                for dc in range(DC):
                    wvt = wkp.tile([128, HD], f32, tag="wvt",
                                   name=f"wvt{dc}")
                    nc.sync.dma_start(
                        r(wvt),
                        r(wv[:].rearrange("(dc p) o -> dc p o", p=128)[dc]))
                    nc.tensor.matmul(psv, r(xT_sb[:, dc, :]), r(wvt),
                                     start=(dc == 0), stop=(dc == DC - 1))

                q_sb = pp.tile([B, OUTW], f32, tag="q_sb")
                nc.vector.tensor_copy(q_sb, psq)
                k_sb = pp.tile([B, HD], f32, tag="k_sb")
                nc.vector.tensor_copy(k_sb, psk)
                vnew_sb = pp.tile([B, HD], f32, tag="vnew_sb")
                nc.vector.tensor_copy(vnew_sb, psv)

                # rope on q (scaled by alpha via csq) and k (unscaled)
                qrot = pp.tile([B, OUTW], f32, tag="qrot")
                tA = qxp.tile([B, OUTW // 2], f32, tag="ropetmp", name="tA")
                tB = qxp.tile([B, OUTW // 2], f32, tag="ropetmp", name="tB")
                qe, qo = q_sb[:, 0::2], q_sb[:, 1::2]
                nc.vector.tensor_mul(tA, qe, cq32)
                nc.vector.tensor_mul(tB, qo, sq32)
                nc.vector.tensor_tensor(qrot[:, 0::2], tA, tB, SUB)
                tC = qxp.tile([B, OUTW // 2], f32, tag="ropetmp", name="tC")
                tD = qxp.tile([B, OUTW // 2], f32, tag="ropetmp", name="tD")
                nc.vector.tensor_mul(tC, qe, sq32)
                nc.vector.tensor_mul(tD, qo, cq32)
                nc.vector.tensor_add(qrot[:, 1::2], tC, tD)

                krot = pp.tile([B, HD], f32, tag="krot")
                uA = qxp.tile([B, HD // 2], f32, tag="kropetmp", name="uA")
                uB = qxp.tile([B, HD // 2], f32, tag="kropetmp", name="uB")
                ke, ko = k_sb[:, 0::2], k_sb[:, 1::2]
                nc.vector.tensor_mul(uA, ke, ck32)
                nc.vector.tensor_mul(uB, ko, sk32)
                nc.vector.tensor_tensor(krot[:, 0::2], uA, uB, SUB)
                uC = qxp.tile([B, HD // 2], f32, tag="kropetmp", name="uC")
                uD = qxp.tile([B, HD // 2], f32, tag="kropetmp", name="uD")
                nc.vector.tensor_mul(uC, ke, sk32)
                nc.vector.tensor_mul(uD, ko, ck32)
                nc.vector.tensor_add(krot[:, 1::2], uC, uD)

                # transpose q (per head) and k_new to [d, b]
                qT_sb = pp.tile([128, HPC, B], f32, tag="qT_sb")
                for h in range(HPC):
                    pst = psA.tile([128, B], f32, tag="pstA",
                                   name=f"pstA{h}")
                    nc.tensor.transpose(pst, qrot[:, h * HD:(h + 1) * HD],
                                        iden_sb[0:B, 0:B])
                    nc.vector.tensor_copy(qT_sb[:, h, :], pst)
                ktnew_sb = pp.tile([128, B], f32, tag="ktnew_sb")
                pstk = psA.tile([128, B], f32, tag="pstA")
                nc.tensor.transpose(pstk, krot, iden_sb[0:B, 0:B])
                nc.vector.tensor_copy(r(ktnew_sb), pstk)
                vnewT_sb = pp.tile([128, B], f32, tag="vnewT_sb")
                pstv = psA.tile([128, B], f32, tag="pstA")
                nc.tensor.transpose(pstv, vnew_sb, iden_sb[0:B, 0:B])
                nc.vector.tensor_copy(vnewT_sb, pstv)

                # zero-padded per-batch q weights [d, bh], batch b in block b
                qxall = pp.tile([128, B * 128], f32, tag="qxall")
                nc.vector.tensor_copy(
                    r(qxall), zero1[:, 0:1].to_broadcast([128, B * 128]))
                for b in range(B):
                    nc.vector.tensor_copy(
                        r(qxall[:, 128 * b + HPC * b:128 * b + HPC * (b + 1)]),
                        qT_sb[:, :, b])

            # ------- phase B: QK scores
            scores = pp.tile([128, T], f32, tag="scores")
            with tc.tile_pool(name="psB", bufs=1, space="PSUM") as psB:
                pqk = [psB.tile([128, 512], f32, tag=f"pqk{c}",
                                name=f"pqk{c}")
                       for c in range(TC)]
                for b in range(B):
                    ktb = ktp.tile([128, T], f32, tag="ktb", name=f"ktb{b}")
                    for q4 in range(4):
                        nc.sync.dma_start(
                            r(ktb[:, q4 * 1024:(q4 + 1) * 1024]),
                            r(kt[b, :, q4 * 1024:(q4 + 1) * 1024]))
                    # patch the new-token k into cache column 4095
                    nc.vector.tensor_copy(r(ktb[:, T - 1:T]),
                                          ktnew_sb[:, b:b + 1])
                    for c in range(TC):
                        nc.tensor.matmul(
                            pqk[c],
                            r(qxall[:, 128 * b:128 * (b + 1)]),
                            r(ktb[:, c * 512:(c + 1) * 512]),
                            start=(b == 0), stop=(b == B - 1))
                for c in range(TC):
                    nc.vector.tensor_copy(scores[:, c * 512:(c + 1) * 512],
                                          pqk[c])
            if dbg:
                nc.sync.dma_start(dbg_qrot[:], qrot)
                nc.sync.dma_start(dbg_krot[:], krot)
                nc.sync.dma_start(dbg_scores[:], scores)

            # ------- softmax (rows = b*4+h on partitions)
            maxv = pp.tile([128, 1], f32, tag="maxv")
            nc.vector.reduce_max(maxv, scores, axis=X)
            negmax = pp.tile([128, 1], f32, tag="negmax")
            nc.vector.tensor_scalar_mul(negmax, maxv, -1.0)
            sums = pp.tile([128, 1], f32, tag="sums")
            nc.scalar.activation(scores, scores, EXP, bias=negmax, scale=1.0,
                                 accum_out=sums)
            recip = pp.tile([128, 1], f32, tag="recip")
            nc.vector.reciprocal(recip, sums)
            nc.vector.tensor_scalar_mul(scores, scores, recip)

            # ------- phase C: transpose p, PV, out projection
            with tc.tile_pool(name="psC", bufs=2, space="PSUM") as psC:
                # extract p[:, 4095] (new-token weights), then zero that
                # column so the stale cache row at t=4095 contributes nothing;
                # the real new-token v is added via the masked correction
                # matmul below
                psr = psC.tile([1, 128], f32, tag="psr", bufs=1)
                nc.tensor.transpose(psr, scores[:, T - 1:T], iden_sb)
                prow = pp.tile([1, 128], f32, tag="prow")
                nc.vector.tensor_copy(r(prow), psr)
                nc.vector.tensor_copy(scores[:, T - 1:T], zero1)
                if dbg:
                    nc.sync.dma_start(dbg_p[:], scores)
                # broadcast prow to all 128 partitions via rank-1 outer product
                psbc = psC.tile([128, 128], f32, tag="psbc", bufs=1)
                nc.tensor.matmul(psbc, r(ones_sb), r(prow))

                pT = pp.tile([128, PC, 128], f32, tag="pT")
                for c2 in range(PC):
                    pstx = psC.tile([128, 128], f32, tag="pstx",
                                    name=f"pstx{c2}")
                    nc.tensor.transpose(pstx,
                                        scores[:, c2 * 128:(c2 + 1) * 128],
                                        iden_sb)
                    nc.vector.tensor_copy(r(pT[:, c2, :]), pstx)
                if dbg:
                    nc.sync.dma_start(dbg_prow[:], prow)

                psat = psC.tile([128, B * HPC], f32, tag="psat", bufs=1)
                for b in range(B):
                    vb = vp.tile([128, PC, HD], f32, tag="vb", name=f"vb{b}")
                    for q4 in range(4):
                        nc.sync.dma_start(r(vb[:, q4 * 8:(q4 + 1) * 8, :]),
                                          r(vc[b, :, q4 * 8:(q4 + 1) * 8, :]))
                    for c2 in range(PC):
                        nc.tensor.matmul(
                            psat[:, HPC * b:HPC * (b + 1)],
                            r(vb[:, c2, :]),
                            r(pT[:, c2, HPC * b:HPC * (b + 1)]),
                            start=(c2 == 0), stop=(c2 == PC - 1),
                            skip_group_check=True)

                # new-token correction on DVE:
                # corrT[d, 4b+h] = vnewT[d, b] * prow[4b+h]
                corrT = pp.tile([128, B, HPC], f32, tag="corrT")
                nc.vector.tensor_mul(
                    corrT,
                    vnewT_sb[:, :, None].to_broadcast([128, B, HPC]),
                    psbc[:].rearrange("d (b h) -> d b h", h=HPC))
                attnT = pp.tile([128, B * HPC], f32, tag="attnT")
                if corr:
                    nc.vector.tensor_add(
                        r(attnT), psat,
                        corrT[:].rearrange("d b h -> d (b h)"))
                else:
                    nc.vector.tensor_copy(r(attnT), psat)
                if dbg:
                    nc.sync.dma_start(dbg_attnT[:], attnT)

                out_sb = pp.tile([B, DIM], f32, tag="out_sb")
                for ncc in range(8):
                    pso = psC.tile([B, 512], f32, tag="pso", name=f"pso{ncc}")
                    for h in range(HPC):
                        wot = wp.tile([128, 512], f32, tag="wchunk",
                                      name=f"wot{ncc}_{h}")
                        nc.sync.dma_start(
                            r(wot), r(wo[h * HD:(h + 1) * HD,
                                         ncc * 512:(ncc + 1) * 512]))
                        nc.tensor.matmul(pso, r(attnT[:, h::HPC]), r(wot),
                                         start=(h == 0), stop=(h == HPC - 1))
                    nc.vector.tensor_copy(out_sb[:, ncc * 512:(ncc + 1) * 512],
                                          pso)
                nc.sync.dma_start(outp[:], out_sb)

    nc.compile()
    return nc


def make_in_maps(inputs):
    x = np.asarray(inputs["x"], np.float32).reshape(B, DIM)
    cache_k = np.asarray(inputs["cache_k"], np.float32)
    cache_v = np.asarray(inputs["cache_v"], np.float32)
    wq = np.asarray(inputs["wq"], np.float32)
    wk = np.asarray(inputs["wk"], np.float32)
    wv = np.asarray(inputs["wv"], np.float32)
    wo = np.asarray(inputs["wo"], np.float32)
    cos = np.asarray(inputs["freqs_cos"], np.float32).reshape(-1)
    sin = np.asarray(inputs["freqs_sin"], np.float32).reshape(-1)

    xT = np.ascontiguousarray(x.T)                             # [DIM, B]
    csq = np.ascontiguousarray(
        np.stack([np.tile(cos, HPC), np.tile(sin, HPC)]) * ALPHA)
    csk = np.ascontiguousarray(np.stack([cos, sin]))
    onesv = np.ones((1, 128), np.float32)
    iden = np.eye(128, dtype=np.float32)

    in_maps = []
    for g in range(NCORES):
        kt_g = np.ascontiguousarray(
            cache_k[:, :, g, :].transpose(0, 2, 1))            # [B, HD, T]
        v_g = np.ascontiguousarray(
            cache_v[:, :, g, :].reshape(B, PC, 128, HD)
            .transpose(0, 2, 1, 3))                            # [B, 128, PC, HD]
        in_maps.append({
            "xT": xT,
            "wq": np.ascontiguousarray(wq[:, g * OUTW:(g + 1) * OUTW]),
            "wk": np.ascontiguousarray(wk[:, g * HD:(g + 1) * HD]),
            "wv": np.ascontiguousarray(wv[:, g * HD:(g + 1) * HD]),
            "wo": np.ascontiguousarray(wo[g * OUTW:(g + 1) * OUTW, :]),
            "kt": kt_g,
            "vc": v_g,
            "csq": csq,
            "csk": csk,
            "ones": onesv,
            "iden": iden,
        })
    return in_maps


_NC_CACHE = []


def run(inputs, trace=False, **kwargs):
    from concourse.bass_utils import run_bass_kernel_spmd
    if not _NC_CACHE:
        _NC_CACHE.append(build_nc())
    nc = _NC_CACHE[0]
    in_maps = make_in_maps(inputs)
    res = run_bass_kernel_spmd(nc, in_maps, core_ids=list(range(NCORES)),
                               trace=trace, **kwargs)
    partials = np.stack([r["outp"] for r in res.results])      # [8, B, DIM]
    out = partials.sum(axis=0, dtype=np.float64).astype(np.float32)
    return out.reshape(B, 1, DIM), res


def kernel(**inputs):
    out, _ = run(inputs)
    return out


# revision 30
# speedup vs baseline: 1.0014x; 1.0014x over previous
"""GQA decode attention (B=32, S=1, 32 Q heads / 8 KV heads, HD=128, T=4096)
for 8 Trainium2 NeuronCores, tensor-parallel over heads.

Per core g: 4 query heads (4g..4g+3) + KV head g.
  - QKV projections from x (contraction over DIM=4096 on PE, fp32r)
  - RoPE on q/k (DVE, strided even/odd APs), 1/sqrt(HD) folded into q's rope
  - scores[bh, t] via zero-padded per-batch q weights (one PSUM bank per
    512-wide T chunk, accumulated over the 32 batches)
  - new-token k patched into the streamed K^T tile column 4095
  - softmax over the free axis on full 128 partitions (b*4+h), exp on ACT
    with fused accumulated row-sums; p normalized in place
  - p transposed per 128-chunk on PE; PV with V as stationary operand,
    output [d, bh] accumulated in one PSUM bank; new-token v added via one
    masked rank-32 matmul correction
  - out projection wo with attnT column-strided weights

Host side: shards weights/caches per head, pre-transposes K cache to
[B, HD, T] and pre-swizzles V cache to [B, 128, 32, HD] so every DMA is
16KB-contiguous per partition. Partial outputs (one per core) summed on host.
"""

import numpy as np

B, DIM, NH, NKV, HD = 32, 4096, 32, 8, 128
T = 4096
NCORES = 8
HPC = NH // NCORES            # 4 query heads per core
OUTW = HPC * HD               # 512
ALPHA = float(1.0 / np.sqrt(HD))
DC = DIM // 128               # 32 contraction chunks for projections
TC = T // 512                 # 8 score chunks (512 wide)
PC = T // 128                 # 32 PV chunks (128 deep)


def build_nc(dbg=False, corr=True):
    import concourse.mybir as mybir
    import concourse.tile as tile
    from concourse import bacc

    f32 = mybir.dt.float32
    f32r = mybir.dt.float32r
    X = mybir.AxisListType.X
    EXP = mybir.ActivationFunctionType.Exp
    SUB = mybir.AluOpType.subtract

    nc = bacc.Bacc("TRN2", target_bir_lowering=False, debug=False,
                   num_devices=NCORES)

    xT = nc.dram_tensor("xT", [DIM, B], f32, kind="ExternalInput")
    wq = nc.dram_tensor("wq", [DIM, OUTW], f32, kind="ExternalInput")
    wk = nc.dram_tensor("wk", [DIM, HD], f32, kind="ExternalInput")
    wv = nc.dram_tensor("wv", [DIM, HD], f32, kind="ExternalInput")
    wo = nc.dram_tensor("wo", [OUTW, DIM], f32, kind="ExternalInput")
    kt = nc.dram_tensor("kt", [B, HD, T], f32, kind="ExternalInput")
    vc = nc.dram_tensor("vc", [B, 128, PC, HD], f32, kind="ExternalInput")
    csq = nc.dram_tensor("csq", [2, OUTW // 2], f32, kind="ExternalInput")
    csk = nc.dram_tensor("csk", [2, HD // 2], f32, kind="ExternalInput")
    ones = nc.dram_tensor("ones", [1, 128], f32, kind="ExternalInput")
    iden = nc.dram_tensor("iden", [128, 128], f32, kind="ExternalInput")
    outp = nc.dram_tensor("outp", [B, DIM], f32, kind="ExternalOutput")
    if dbg:
        dbg_qrot = nc.dram_tensor("dbg_qrot", [B, OUTW], f32,
                                  kind="ExternalOutput")
        dbg_krot = nc.dram_tensor("dbg_krot", [B, HD], f32,
                                  kind="ExternalOutput")
        dbg_scores = nc.dram_tensor("dbg_scores", [128, T], f32,
                                    kind="ExternalOutput")
        dbg_p = nc.dram_tensor("dbg_p", [128, T], f32, kind="ExternalOutput")
        dbg_attnT = nc.dram_tensor("dbg_attnT", [128, B * HPC], f32,
                                   kind="ExternalOutput")
        dbg_prow = nc.dram_tensor("dbg_prow", [1, 128], f32,
                                  kind="ExternalOutput")

    def r(ap):
        return ap.bitcast(f32r)

    with tile.TileContext(nc) as tc:
        with (
            tc.tile_pool(name="pp", bufs=1) as pp,
            tc.tile_pool(name="wp", bufs=4) as wp,
            tc.tile_pool(name="wkp", bufs=3) as wkp,
            tc.tile_pool(name="ktp", bufs=3) as ktp,
            tc.tile_pool(name="vp", bufs=3) as vp,
            tc.tile_pool(name="qxp", bufs=2) as qxp,
            tc.tile_pool(name="outp_pool", bufs=2) as outpp,
        ):
            # ------- constants & persistent tiles
            xT_sb = pp.tile([128, DC, B], f32, tag="xT_sb")
            nc.sync.dma_start(r(xT_sb),
                              r(xT[:].rearrange("(dc p) b -> p dc b", p=128)))
            iden_sb = pp.tile([128, 128], f32, tag="iden_sb")
            nc.sync.dma_start(iden_sb, iden[:])
            ones_sb = pp.tile([1, 128], f32, tag="ones_sb")
            nc.sync.dma_start(r(ones_sb), r(ones[:]))
            cq32 = pp.tile([B, OUTW // 2], f32, tag="cq32")
            nc.sync.dma_start(cq32, csq[0:1, :].to_broadcast([B, OUTW // 2]))
            sq32 = pp.tile([B, OUTW // 2], f32, tag="sq32")
            nc.sync.dma_start(sq32, csq[1:2, :].to_broadcast([B, OUTW // 2]))
            ck32 = pp.tile([B, HD // 2], f32, tag="ck32")
            nc.sync.dma_start(ck32, csk[0:1, :].to_broadcast([B, HD // 2]))
            sk32 = pp.tile([B, HD // 2], f32, tag="sk32")
            nc.sync.dma_start(sk32, csk[1:2, :].to_broadcast([B, HD // 2]))
            zero1 = pp.tile([128, 1], f32, tag="zero1")
            nc.vector.memset(zero1, 0.0)

            # ------- phase A: projections + rope + q/k transposes
            with tc.tile_pool(name="psA", bufs=1, space="PSUM") as psA:
                psq = psA.tile([B, OUTW], f32, tag="psq")
                for dc in range(DC):
                    wqt = wp.tile([128, OUTW], f32, tag="wchunk",
                                  name=f"wqt{dc}")
                    nc.sync.dma_start(
                        r(wqt),
                        r(wq[:].rearrange("(dc p) o -> dc p o", p=128)[dc]))
                    nc.tensor.matmul(psq, r(xT_sb[:, dc, :]), r(wqt),
                                     start=(dc == 0), stop=(dc == DC - 1))
                # fused K|V projection: one 32-matmul chain over [dc] with
                # a [128, 256] weight tile (wk cols then wv cols)
                pskv = psA.tile([B, 2 * HD], f32, tag="pskv")
                for dc in range(DC):
                    wkvt = wkp.tile([128, 2, HD], f32, tag="wkvt",
                                    name=f"wkvt{dc}")
                    nc.sync.dma_start(
                        r(wkvt[:, 0, :]),
                        r(wk[:].rearrange("(dc p) o -> dc p o", p=128)[dc]))
                    nc.sync.dma_start(
                        r(wkvt[:, 1, :]),
                        r(wv[:].rearrange("(dc p) o -> dc p o", p=128)[dc]))
                    nc.tensor.matmul(pskv, r(xT_sb[:, dc, :]),
                                     r(wkvt[:].rearrange("p a o -> p (a o)")),
                                     start=(dc == 0), stop=(dc == DC - 1))

                q_sb = pp.tile([B, OUTW], f32, tag="q_sb")
                nc.vector.tensor_copy(q_sb, psq)
                k_sb = pp.tile([B, HD], f32, tag="k_sb")
                nc.vector.tensor_copy(k_sb, pskv[:, 0:HD])
                vnew_sb = pp.tile([B, HD], f32, tag="vnew_sb")
                nc.vector.tensor_copy(vnew_sb, pskv[:, HD:2 * HD])

                # rope on q (scaled by alpha via csq) and k (unscaled)
                qrot = pp.tile([B, OUTW], f32, tag="qrot")
                tA = qxp.tile([B, OUTW // 2], f32, tag="ropetmp", name="tA")
                tB = qxp.tile([B, OUTW // 2], f32, tag="ropetmp", name="tB")
                qe, qo = q_sb[:, 0::2], q_sb[:, 1::2]
                nc.vector.tensor_mul(tA, qe, cq32)
                nc.vector.tensor_mul(tB, qo, sq32)
                nc.vector.tensor_tensor(qrot[:, 0::2], tA, tB, SUB)
                tC = qxp.tile([B, OUTW // 2], f32, tag="ropetmp", name="tC")
                tD = qxp.tile([B, OUTW // 2], f32, tag="ropetmp", name="tD")
                nc.vector.tensor_mul(tC, qe, sq32)
                nc.vector.tensor_mul(tD, qo, cq32)
                nc.vector.tensor_add(qrot[:, 1::2], tC, tD)

                krot = pp.tile([B, HD], f32, tag="krot")
                uA = qxp.tile([B, HD // 2], f32, tag="kropetmp", name="uA")
                uB = qxp.tile([B, HD // 2], f32, tag="kropetmp", name="uB")
                ke, ko = k_sb[:, 0::2], k_sb[:, 1::2]
                nc.vector.tensor_mul(uA, ke, ck32)
                nc.vector.tensor_mul(uB, ko, sk32)
                nc.vector.tensor_tensor(krot[:, 0::2], uA, uB, SUB)
                uC = qxp.tile([B, HD // 2], f32, tag="kropetmp", name="uC")
                uD = qxp.tile([B, HD // 2], f32, tag="kropetmp", name="uD")
                nc.vector.tensor_mul(uC, ke, sk32)
                nc.vector.tensor_mul(uD, ko, ck32)
                nc.vector.tensor_add(krot[:, 1::2], uC, uD)

                # transpose q (per head) and k_new to [d, b]
                qT_sb = pp.tile([128, HPC, B], f32, tag="qT_sb")
                for h in range(HPC):
                    pst = psA.tile([128, B], f32, tag="pstA",
                                   name=f"pstA{h}")
                    nc.tensor.transpose(pst, qrot[:, h * HD:(h + 1) * HD],
                                        iden_sb[0:B, 0:B])
                    nc.vector.tensor_copy(qT_sb[:, h, :], pst)
                ktnew_sb = pp.tile([128, B], f32, tag="ktnew_sb")
                pstk = psA.tile([128, B], f32, tag="pstA")
                nc.tensor.transpose(pstk, krot, iden_sb[0:B, 0:B])
                nc.vector.tensor_copy(r(ktnew_sb), pstk)
                vnewT_sb = pp.tile([128, B], f32, tag="vnewT_sb")
                pstv = psA.tile([128, B], f32, tag="pstA")
                nc.tensor.transpose(pstv, vnew_sb, iden_sb[0:B, 0:B])
                nc.vector.tensor_copy(vnewT_sb, pstv)

                # zero-padded per-batch q weights [d, bh], batch b in block b
                qxall = pp.tile([128, B * 128], f32, tag="qxall")
                nc.vector.tensor_copy(
                    r(qxall), zero1[:, 0:1].to_broadcast([128, B * 128]))
                for b in range(B):
                    nc.vector.tensor_copy(
                        r(qxall[:, 128 * b + HPC * b:128 * b + HPC * (b + 1)]),
                        qT_sb[:, :, b])

            # ------- phase B: QK scores
            scores = pp.tile([128, T], f32, tag="scores")
            with tc.tile_pool(name="psB", bufs=1, space="PSUM") as psB:
                pqk = [psB.tile([128, 512], f32, tag=f"pqk{c}",
                                name=f"pqk{c}")
                       for c in range(TC)]
                for b in range(B):
                    ktb = ktp.tile([128, T], f32, tag="ktb", name=f"ktb{b}")
                    for q4 in range(4):
                        nc.sync.dma_start(
                            r(ktb[:, q4 * 1024:(q4 + 1) * 1024]),
                            r(kt[b, :, q4 * 1024:(q4 + 1) * 1024]))
                    # patch the new-token k into cache column 4095
                    nc.vector.tensor_copy(r(ktb[:, T - 1:T]),
                                          ktnew_sb[:, b:b + 1])
                    for c in range(TC):
                        nc.tensor.matmul(
                            pqk[c],
                            r(qxall[:, 128 * b:128 * (b + 1)]),
                            r(ktb[:, c * 512:(c + 1) * 512]),
                            start=(b == 0), stop=(b == B - 1))
                for c in range(TC):
                    nc.vector.tensor_copy(scores[:, c * 512:(c + 1) * 512],
                                          pqk[c])
            if dbg:
                nc.sync.dma_start(dbg_qrot[:], qrot)
                nc.sync.dma_start(dbg_krot[:], krot)
                nc.sync.dma_start(dbg_scores[:], scores)

            # ------- softmax (rows = b*4+h on partitions)
            maxv = pp.tile([128, 1], f32, tag="maxv")
            nc.vector.reduce_max(maxv, scores, axis=X)
            negmax = pp.tile([128, 1], f32, tag="negmax")
            nc.vector.tensor_scalar_mul(negmax, maxv, -1.0)
            sums = pp.tile([128, 1], f32, tag="sums")
            nc.scalar.activation(scores, scores, EXP, bias=negmax, scale=1.0,
                                 accum_out=sums)
            recip = pp.tile([128, 1], f32, tag="recip")
            nc.vector.reciprocal(recip, sums)
            nc.vector.tensor_scalar_mul(scores, scores, recip)

            # ------- phase C: transpose p, PV, out projection
            with tc.tile_pool(name="psC", bufs=2, space="PSUM") as psC:
                # extract p[:, 4095] (new-token weights), then zero that
                # column so the stale cache row at t=4095 contributes nothing;
                # the real new-token v is added via the masked correction
                # matmul below
                psr = psC.tile([1, 128], f32, tag="psr", bufs=1)
                nc.tensor.transpose(psr, scores[:, T - 1:T], iden_sb)
                prow = pp.tile([1, 128], f32, tag="prow")
                nc.vector.tensor_copy(r(prow), psr)
                nc.vector.tensor_copy(scores[:, T - 1:T], zero1)
                if dbg:
                    nc.sync.dma_start(dbg_p[:], scores)
                # broadcast prow to all 128 partitions via rank-1 outer product
                psbc = psC.tile([128, 128], f32, tag="psbc", bufs=1)
                nc.tensor.matmul(psbc, r(ones_sb), r(prow))

                pT = pp.tile([128, PC, 128], f32, tag="pT")
                for c2 in range(PC):
                    pstx = psC.tile([128, 128], f32, tag="pstx",
                                    name=f"pstx{c2}")
                    nc.tensor.transpose(pstx,
                                        scores[:, c2 * 128:(c2 + 1) * 128],
                                        iden_sb)
                    nc.vector.tensor_copy(r(pT[:, c2, :]), pstx)
                if dbg:
                    nc.sync.dma_start(dbg_prow[:], prow)

                psat = psC.tile([128, B * HPC], f32, tag="psat", bufs=1)
                for b in range(B):
                    vb = vp.tile([128, PC, HD], f32, tag="vb", name=f"vb{b}")
                    for q4 in range(4):
                        nc.sync.dma_start(r(vb[:, q4 * 8:(q4 + 1) * 8, :]),
                                          r(vc[b, :, q4 * 8:(q4 + 1) * 8, :]))
                    for c2 in range(PC):
                        nc.tensor.matmul(
                            psat[:, HPC * b:HPC * (b + 1)],
                            r(vb[:, c2, :]),
                            r(pT[:, c2, HPC * b:HPC * (b + 1)]),
                            start=(c2 == 0), stop=(c2 == PC - 1),
                            skip_group_check=True)

                # new-token correction on DVE:
                # corrT[d, 4b+h] = vnewT[d, b] * prow[4b+h]
                corrT = pp.tile([128, B, HPC], f32, tag="corrT")
                nc.vector.tensor_mul(
                    corrT,
                    vnewT_sb[:, :, None].to_broadcast([128, B, HPC]),
                    psbc[:].rearrange("d (b h) -> d b h", h=HPC))
                attnT = pp.tile([128, B * HPC], f32, tag="attnT")
                if corr:
                    nc.vector.tensor_add(
                        r(attnT), psat,
                        corrT[:].rearrange("d b h -> d (b h)"))
                else:
                    nc.vector.tensor_copy(r(attnT), psat)
                if dbg:
                    nc.sync.dma_start(dbg_attnT[:], attnT)

                for ncc in range(8):
                    pso = psC.tile([B, 512], f32, tag="pso", name=f"pso{ncc}")
                    for h in range(HPC):
                        wot = wp.tile([128, 512], f32, tag="wchunk",
                                      name=f"wot{ncc}_{h}")
                        nc.sync.dma_start(
                            r(wot), r(wo[h * HD:(h + 1) * HD,
                                         ncc * 512:(ncc + 1) * 512]))
                        nc.tensor.matmul(pso, r(attnT[:, h::HPC]), r(wot),
                                         start=(h == 0), stop=(h == HPC - 1))
                    osb = outpp.tile([B, 512], f32, tag="osb",
                                     name=f"osb{ncc}")
                    nc.vector.tensor_copy(osb, pso)
                    nc.sync.dma_start(outp[:, ncc * 512:(ncc + 1) * 512], osb)

    nc.compile()
    return nc


def make_in_maps(inputs):
    x = np.asarray(inputs["x"], np.float32).reshape(B, DIM)
    cache_k = np.asarray(inputs["cache_k"], np.float32)
    cache_v = np.asarray(inputs["cache_v"], np.float32)
    wq = np.asarray(inputs["wq"], np.float32)
    wk = np.asarray(inputs["wk"], np.float32)
    wv = np.asarray(inputs["wv"], np.float32)
    wo = np.asarray(inputs["wo"], np.float32)
    cos = np.asarray(inputs["freqs_cos"], np.float32).reshape(-1)
    sin = np.asarray(inputs["freqs_sin"], np.float32).reshape(-1)

    xT = np.ascontiguousarray(x.T)                             # [DIM, B]
    csq = np.ascontiguousarray(
        np.stack([np.tile(cos, HPC), np.tile(sin, HPC)]) * ALPHA)
    csk = np.ascontiguousarray(np.stack([cos, sin]))
    onesv = np.ones((1, 128), np.float32)
    iden = np.eye(128, dtype=np.float32)

    in_maps = []
    for g in range(NCORES):
        kt_g = np.ascontiguousarray(
            cache_k[:, :, g, :].transpose(0, 2, 1))            # [B, HD, T]
        v_g = np.ascontiguousarray(
            cache_v[:, :, g, :].reshape(B, PC, 128, HD)
            .transpose(0, 2, 1, 3))                            # [B, 128, PC, HD]
        in_maps.append({
            "xT": xT,
            "wq": np.ascontiguousarray(wq[:, g * OUTW:(g + 1) * OUTW]),
            "wk": np.ascontiguousarray(wk[:, g * HD:(g + 1) * HD]),
            "wv": np.ascontiguousarray(wv[:, g * HD:(g + 1) * HD]),
            "wo": np.ascontiguousarray(wo[g * OUTW:(g + 1) * OUTW, :]),
            "kt": kt_g,
            "vc": v_g,
            "csq": csq,
            "csk": csk,
            "ones": onesv,
            "iden": iden,
        })
    return in_maps


_NC_CACHE = []


def run(inputs, trace=False, **kwargs):
    from concourse.bass_utils import run_bass_kernel_spmd
    if not _NC_CACHE:
        _NC_CACHE.append(build_nc())
    nc = _NC_CACHE[0]
    in_maps = make_in_maps(inputs)
    res = run_bass_kernel_spmd(nc, in_maps, core_ids=list(range(NCORES)),
                               trace=trace, **kwargs)
    partials = np.stack([r["outp"] for r in res.results])      # [8, B, DIM]
    out = partials.sum(axis=0, dtype=np.float64).astype(np.float32)
    return out.reshape(B, 1, DIM), res


def kernel(**inputs):
    out, _ = run(inputs)
    return out


# revision 34
# speedup vs baseline: 1.5027x; 1.5006x over previous
"""GQA decode attention (B=32, S=1, 32 Q heads / 8 KV heads, HD=128, T=4096)
for 8 Trainium2 NeuronCores, tensor-parallel over heads.

Per core g: 4 query heads (4g..4g+3) + KV head g.
  - QKV projections from x (contraction over DIM=4096 on PE, fp32r)
  - RoPE on q/k (DVE, strided even/odd APs), 1/sqrt(HD) folded into q's rope
  - scores[bh, t] via zero-padded per-batch q weights (one PSUM bank per
    512-wide T chunk, accumulated over the 32 batches)
  - new-token k patched into the streamed K^T tile column 4095
  - softmax over the free axis on full 128 partitions (b*4+h), exp on ACT
    with fused accumulated row-sums; p normalized in place
  - p transposed per 128-chunk on PE; PV with V as stationary operand,
    output [d, bh] accumulated in one PSUM bank; new-token v added via one
    masked rank-32 matmul correction
  - out projection wo with attnT column-strided weights

Host side: shards weights/caches per head, pre-transposes K cache to
[B, HD, T] and pre-swizzles V cache to [B, 128, 32, HD] so every DMA is
16KB-contiguous per partition. Partial outputs (one per core) summed on host.
"""

import numpy as np

B, DIM, NH, NKV, HD = 32, 4096, 32, 8, 128
T = 4096
NCORES = 8
HPC = NH // NCORES            # 4 query heads per core
OUTW = HPC * HD               # 512
ALPHA = float(1.0 / np.sqrt(HD))
DC = DIM // 128               # 32 contraction chunks for projections
TC = T // 512                 # 8 score chunks (512 wide)
PC = T // 128                 # 32 PV chunks (128 deep)


KV_BF16 = True


def build_nc(dbg=False, corr=True, kv_bf16=KV_BF16):
    import concourse.mybir as mybir
    import concourse.tile as tile
    from concourse import bacc

    f32 = mybir.dt.float32
    f32r = mybir.dt.float32r
    bf16 = mybir.dt.bfloat16
    kvdt = bf16 if kv_bf16 else f32
    X = mybir.AxisListType.X
    EXP = mybir.ActivationFunctionType.Exp
    SUB = mybir.AluOpType.subtract

    nc = bacc.Bacc("TRN2", target_bir_lowering=False, debug=False,
                   num_devices=NCORES)

    xT = nc.dram_tensor("xT", [DIM, B], f32, kind="ExternalInput")
    wq = nc.dram_tensor("wq", [DIM, OUTW], f32, kind="ExternalInput")
    wk = nc.dram_tensor("wk", [DIM, HD], f32, kind="ExternalInput")
    wv = nc.dram_tensor("wv", [DIM, HD], f32, kind="ExternalInput")
    wo = nc.dram_tensor("wo", [OUTW, DIM], f32, kind="ExternalInput")
    kt = nc.dram_tensor("kt", [B, HD, T], kvdt, kind="ExternalInput")
    vc = nc.dram_tensor("vc", [B, 128, PC, HD], kvdt, kind="ExternalInput")
    csq = nc.dram_tensor("csq", [2, OUTW // 2], f32, kind="ExternalInput")
    csk = nc.dram_tensor("csk", [2, HD // 2], f32, kind="ExternalInput")
    ones = nc.dram_tensor("ones", [1, 128], f32, kind="ExternalInput")
    iden = nc.dram_tensor("iden", [128, 128], f32, kind="ExternalInput")
    outp = nc.dram_tensor("outp", [B, DIM], f32, kind="ExternalOutput")
    if dbg:
        dbg_qrot = nc.dram_tensor("dbg_qrot", [B, OUTW], f32,
                                  kind="ExternalOutput")
        dbg_krot = nc.dram_tensor("dbg_krot", [B, HD], f32,
                                  kind="ExternalOutput")
        dbg_scores = nc.dram_tensor("dbg_scores", [128, T], f32,
                                    kind="ExternalOutput")
        dbg_p = nc.dram_tensor("dbg_p", [128, T], f32, kind="ExternalOutput")
        dbg_attnT = nc.dram_tensor("dbg_attnT", [128, B * HPC], f32,
                                   kind="ExternalOutput")
        dbg_prow = nc.dram_tensor("dbg_prow", [1, 128], f32,
                                  kind="ExternalOutput")

    def r(ap):
        return ap.bitcast(f32r)

    def rkv(ap):
        # KV-path matmul operand view: bf16 tiles pass through, f32 tiles
        # are viewed as f32r
        return ap if kv_bf16 else ap.bitcast(f32r)

    kv_bufs = 5 if kv_bf16 else 3

    with tile.TileContext(nc) as tc:
        with (
            tc.tile_pool(name="pp", bufs=1) as pp,
            tc.tile_pool(name="wp", bufs=4) as wp,
            tc.tile_pool(name="wkp", bufs=3) as wkp,
            tc.tile_pool(name="ktp", bufs=kv_bufs) as ktp,
            tc.tile_pool(name="vp", bufs=kv_bufs) as vp,
            tc.tile_pool(name="qxp", bufs=2) as qxp,
            tc.tile_pool(name="outp_pool", bufs=2) as outpp,
        ):
            # ------- constants & persistent tiles
            xT_sb = pp.tile([128, DC, B], f32, tag="xT_sb")
            nc.sync.dma_start(r(xT_sb),
                              r(xT[:].rearrange("(dc p) b -> p dc b", p=128)))
            iden_sb = pp.tile([128, 128], f32, tag="iden_sb")
            nc.sync.dma_start(iden_sb, iden[:])
            ones_sb = pp.tile([1, 128], f32, tag="ones_sb")
            nc.sync.dma_start(r(ones_sb), r(ones[:]))
            cq32 = pp.tile([B, OUTW // 2], f32, tag="cq32")
            nc.sync.dma_start(cq32, csq[0:1, :].to_broadcast([B, OUTW // 2]))
            sq32 = pp.tile([B, OUTW // 2], f32, tag="sq32")
            nc.sync.dma_start(sq32, csq[1:2, :].to_broadcast([B, OUTW // 2]))
            ck32 = pp.tile([B, HD // 2], f32, tag="ck32")
            nc.sync.dma_start(ck32, csk[0:1, :].to_broadcast([B, HD // 2]))
            sk32 = pp.tile([B, HD // 2], f32, tag="sk32")
            nc.sync.dma_start(sk32, csk[1:2, :].to_broadcast([B, HD // 2]))
            zero1 = pp.tile([128, 1], f32, tag="zero1")
            nc.vector.memset(zero1, 0.0)

            # ------- phase A: projections + rope + q/k transposes
            with tc.tile_pool(name="psA", bufs=1, space="PSUM") as psA:
                psq = psA.tile([B, OUTW], f32, tag="psq")
                for dc in range(DC):
                    wqt = wp.tile([128, OUTW], f32, tag="wchunk",
                                  name=f"wqt{dc}")
                    nc.sync.dma_start(
                        r(wqt),
                        r(wq[:].rearrange("(dc p) o -> dc p o", p=128)[dc]))
                    nc.tensor.matmul(psq, r(xT_sb[:, dc, :]), r(wqt),
                                     start=(dc == 0), stop=(dc == DC - 1))
                # fused K|V projection: one 32-matmul chain over [dc] with
                # a [128, 256] weight tile (wk cols then wv cols)
                pskv = psA.tile([B, 2 * HD], f32, tag="pskv")
                for dc in range(DC):
                    wkvt = wkp.tile([128, 2, HD], f32, tag="wkvt",
                                    name=f"wkvt{dc}")
                    nc.sync.dma_start(
                        r(wkvt[:, 0, :]),
                        r(wk[:].rearrange("(dc p) o -> dc p o", p=128)[dc]))
                    nc.sync.dma_start(
                        r(wkvt[:, 1, :]),
                        r(wv[:].rearrange("(dc p) o -> dc p o", p=128)[dc]))
                    nc.tensor.matmul(pskv, r(xT_sb[:, dc, :]),
                                     r(wkvt[:].rearrange("p a o -> p (a o)")),
                                     start=(dc == 0), stop=(dc == DC - 1))

                q_sb = pp.tile([B, OUTW], f32, tag="q_sb")
                nc.vector.tensor_copy(q_sb, psq)
                k_sb = pp.tile([B, HD], f32, tag="k_sb")
                nc.vector.tensor_copy(k_sb, pskv[:, 0:HD])
                vnew_sb = pp.tile([B, HD], f32, tag="vnew_sb")
                nc.vector.tensor_copy(vnew_sb, pskv[:, HD:2 * HD])

                # rope on q (scaled by alpha via csq) and k (unscaled)
                qrot = pp.tile([B, OUTW], f32, tag="qrot")
                tA = qxp.tile([B, OUTW // 2], f32, tag="ropetmp", name="tA")
                tB = qxp.tile([B, OUTW // 2], f32, tag="ropetmp", name="tB")
                qe, qo = q_sb[:, 0::2], q_sb[:, 1::2]
                nc.vector.tensor_mul(tA, qe, cq32)
                nc.vector.tensor_mul(tB, qo, sq32)
                nc.vector.tensor_tensor(qrot[:, 0::2], tA, tB, SUB)
                tC = qxp.tile([B, OUTW // 2], f32, tag="ropetmp", name="tC")
                tD = qxp.tile([B, OUTW // 2], f32, tag="ropetmp", name="tD")
                nc.vector.tensor_mul(tC, qe, sq32)
                nc.vector.tensor_mul(tD, qo, cq32)
                nc.vector.tensor_add(qrot[:, 1::2], tC, tD)

                krot = pp.tile([B, HD], f32, tag="krot")
                uA = qxp.tile([B, HD // 2], f32, tag="kropetmp", name="uA")
                uB = qxp.tile([B, HD // 2], f32, tag="kropetmp", name="uB")
                ke, ko = k_sb[:, 0::2], k_sb[:, 1::2]
                nc.vector.tensor_mul(uA, ke, ck32)
                nc.vector.tensor_mul(uB, ko, sk32)
                nc.vector.tensor_tensor(krot[:, 0::2], uA, uB, SUB)
                uC = qxp.tile([B, HD // 2], f32, tag="kropetmp", name="uC")
                uD = qxp.tile([B, HD // 2], f32, tag="kropetmp", name="uD")
                nc.vector.tensor_mul(uC, ke, sk32)
                nc.vector.tensor_mul(uD, ko, ck32)
                nc.vector.tensor_add(krot[:, 1::2], uC, uD)

                # transpose q (per head) and k_new to [d, b]
                qT_sb = pp.tile([128, HPC, B], f32, tag="qT_sb")
                for h in range(HPC):
                    pst = psA.tile([128, B], f32, tag="pstA",
                                   name=f"pstA{h}")
                    nc.tensor.transpose(pst, qrot[:, h * HD:(h + 1) * HD],
                                        iden_sb[0:B, 0:B])
                    nc.vector.tensor_copy(qT_sb[:, h, :], pst)
                ktnew_sb = pp.tile([128, B], kvdt, tag="ktnew_sb")
                pstk = psA.tile([128, B], f32, tag="pstA")
                nc.tensor.transpose(pstk, krot, iden_sb[0:B, 0:B])
                nc.vector.tensor_copy(rkv(ktnew_sb), pstk)
                vnewT_sb = pp.tile([128, B], f32, tag="vnewT_sb")
                pstv = psA.tile([128, B], f32, tag="pstA")
                nc.tensor.transpose(pstv, vnew_sb, iden_sb[0:B, 0:B])
                nc.vector.tensor_copy(vnewT_sb, pstv)

                # zero-padded per-batch q weights [d, bh], batch b in block b
                qxall = pp.tile([128, B * 128], kvdt, tag="qxall")
                nc.vector.tensor_copy(
                    rkv(qxall), zero1[:, 0:1].to_broadcast([128, B * 128]))
                for b in range(B):
                    nc.vector.tensor_copy(
                        rkv(qxall[:, 128 * b + HPC * b:128 * b
                                  + HPC * (b + 1)]),
                        qT_sb[:, :, b])

            # ------- phase B: QK scores
            scores = pp.tile([128, T], f32, tag="scores")
            with tc.tile_pool(name="psB", bufs=1, space="PSUM") as psB:
                pqk = [psB.tile([128, 512], f32, tag=f"pqk{c}",
                                name=f"pqk{c}")
                       for c in range(TC)]
                for b in range(B):
                    ktb = ktp.tile([128, T], kvdt, tag="ktb", name=f"ktb{b}")
                    for q4 in range(4):
                        nc.sync.dma_start(
                            rkv(ktb[:, q4 * 1024:(q4 + 1) * 1024]),
                            rkv(kt[b, :, q4 * 1024:(q4 + 1) * 1024]))
                    # patch the new-token k into cache column 4095
                    nc.vector.tensor_copy(rkv(ktb[:, T - 1:T]),
                                          ktnew_sb[:, b:b + 1])
                    for c in range(TC):
                        nc.tensor.matmul(
                            pqk[c],
                            rkv(qxall[:, 128 * b:128 * (b + 1)]),
                            rkv(ktb[:, c * 512:(c + 1) * 512]),
                            start=(b == 0), stop=(b == B - 1))
                for c in range(TC):
                    nc.vector.tensor_copy(scores[:, c * 512:(c + 1) * 512],
                                          pqk[c])
            if dbg:
                nc.sync.dma_start(dbg_qrot[:], qrot)
                nc.sync.dma_start(dbg_krot[:], krot)
                nc.sync.dma_start(dbg_scores[:], scores)

            # ------- softmax (rows = b*4+h on partitions)
            maxv = pp.tile([128, 1], f32, tag="maxv")
            nc.vector.reduce_max(maxv, scores, axis=X)
            negmax = pp.tile([128, 1], f32, tag="negmax")
            nc.vector.tensor_scalar_mul(negmax, maxv, -1.0)
            sums = pp.tile([128, 1], f32, tag="sums")
            nc.scalar.activation(scores, scores, EXP, bias=negmax, scale=1.0,
                                 accum_out=sums)
            recip = pp.tile([128, 1], f32, tag="recip")
            nc.vector.reciprocal(recip, sums)
            nc.vector.tensor_scalar_mul(scores, scores, recip)

            # ------- phase C: transpose p, PV, out projection
            with tc.tile_pool(name="psC", bufs=2, space="PSUM") as psC:
                # extract p[:, 4095] (new-token weights), then zero that
                # column so the stale cache row at t=4095 contributes nothing;
                # the real new-token v is added via the masked correction
                # matmul below
                psr = psC.tile([1, 128], f32, tag="psr", bufs=1)
                nc.tensor.transpose(psr, scores[:, T - 1:T], iden_sb)
                prow = pp.tile([1, 128], f32, tag="prow")
                nc.vector.tensor_copy(r(prow), psr)
                nc.vector.tensor_copy(scores[:, T - 1:T], zero1)
                if dbg:
                    nc.sync.dma_start(dbg_p[:], scores)
                # broadcast prow to all 128 partitions via rank-1 outer product
                psbc = psC.tile([128, 128], f32, tag="psbc", bufs=1)
                nc.tensor.matmul(psbc, r(ones_sb), r(prow))

                pT = pp.tile([128, PC, 128], kvdt, tag="pT")
                for c2 in range(PC):
                    pstx = psC.tile([128, 128], f32, tag="pstx",
                                    name=f"pstx{c2}")
                    nc.tensor.transpose(pstx,
                                        scores[:, c2 * 128:(c2 + 1) * 128],
                                        iden_sb)
                    nc.vector.tensor_copy(rkv(pT[:, c2, :]), pstx)
                if dbg:
                    nc.sync.dma_start(dbg_prow[:], prow)

                psat = psC.tile([128, B * HPC], f32, tag="psat", bufs=1)
                for b in range(B):
                    vb = vp.tile([128, PC, HD], kvdt, tag="vb", name=f"vb{b}")
                    for q4 in range(4):
                        nc.sync.dma_start(rkv(vb[:, q4 * 8:(q4 + 1) * 8, :]),
                                          rkv(vc[b, :, q4 * 8:(q4 + 1) * 8, :]))
                    for c2 in range(PC):
                        nc.tensor.matmul(
                            psat[:, HPC * b:HPC * (b + 1)],
                            rkv(vb[:, c2, :]),
                            rkv(pT[:, c2, HPC * b:HPC * (b + 1)]),
                            start=(c2 == 0), stop=(c2 == PC - 1),
                            skip_group_check=True)

                # new-token correction on DVE:
                # corrT[d, 4b+h] = vnewT[d, b] * prow[4b+h]
                corrT = pp.tile([128, B, HPC], f32, tag="corrT")
                nc.vector.tensor_mul(
                    corrT,
                    vnewT_sb[:, :, None].to_broadcast([128, B, HPC]),
                    psbc[:].rearrange("d (b h) -> d b h", h=HPC))
                attnT = pp.tile([128, B * HPC], f32, tag="attnT")
                if corr:
                    nc.vector.tensor_add(
                        r(attnT), psat,
                        corrT[:].rearrange("d b h -> d (b h)"))
                else:
                    nc.vector.tensor_copy(r(attnT), psat)
                if dbg:
                    nc.sync.dma_start(dbg_attnT[:], attnT)

                for ncc in range(8):
                    pso = psC.tile([B, 512], f32, tag="pso", name=f"pso{ncc}")
                    for h in range(HPC):
                        wot = wp.tile([128, 512], f32, tag="wchunk",
                                      name=f"wot{ncc}_{h}")
                        nc.sync.dma_start(
                            r(wot), r(wo[h * HD:(h + 1) * HD,
                                         ncc * 512:(ncc + 1) * 512]))
                        nc.tensor.matmul(pso, r(attnT[:, h::HPC]), r(wot),
                                         start=(h == 0), stop=(h == HPC - 1))
                    osb = outpp.tile([B, 512], f32, tag="osb",
                                     name=f"osb{ncc}")
                    nc.vector.tensor_copy(osb, pso)
                    nc.sync.dma_start(outp[:, ncc * 512:(ncc + 1) * 512], osb)

    nc.compile()
    return nc


def make_in_maps(inputs):
    x = np.asarray(inputs["x"], np.float32).reshape(B, DIM)
    cache_k = np.asarray(inputs["cache_k"], np.float32)
    cache_v = np.asarray(inputs["cache_v"], np.float32)
    wq = np.asarray(inputs["wq"], np.float32)
    wk = np.asarray(inputs["wk"], np.float32)
    wv = np.asarray(inputs["wv"], np.float32)
    wo = np.asarray(inputs["wo"], np.float32)
    cos = np.asarray(inputs["freqs_cos"], np.float32).reshape(-1)
    sin = np.asarray(inputs["freqs_sin"], np.float32).reshape(-1)

    xT = np.ascontiguousarray(x.T)                             # [DIM, B]
    csq = np.ascontiguousarray(
        np.stack([np.tile(cos, HPC), np.tile(sin, HPC)]) * ALPHA)
    csk = np.ascontiguousarray(np.stack([cos, sin]))
    onesv = np.ones((1, 128), np.float32)
    iden = np.eye(128, dtype=np.float32)

    kv_np = np.float32
    if KV_BF16:
        import ml_dtypes
        kv_np = ml_dtypes.bfloat16

    in_maps = []
    for g in range(NCORES):
        kt_g = np.ascontiguousarray(
            cache_k[:, :, g, :].transpose(0, 2, 1)).astype(kv_np)  # [B,HD,T]
        v_g = np.ascontiguousarray(
            cache_v[:, :, g, :].reshape(B, PC, 128, HD)
            .transpose(0, 2, 1, 3)).astype(kv_np)              # [B,128,PC,HD]
        in_maps.append({
            "xT": xT,
            "wq": np.ascontiguousarray(wq[:, g * OUTW:(g + 1) * OUTW]),
            "wk": np.ascontiguousarray(wk[:, g * HD:(g + 1) * HD]),
            "wv": np.ascontiguousarray(wv[:, g * HD:(g + 1) * HD]),
            "wo": np.ascontiguousarray(wo[g * OUTW:(g + 1) * OUTW, :]),
            "kt": kt_g,
            "vc": v_g,
            "csq": csq,
            "csk": csk,
            "ones": onesv,
            "iden": iden,
        })
    return in_maps


_NC_CACHE = []


def run(inputs, trace=False, **kwargs):
    from concourse.bass_utils import run_bass_kernel_spmd
    if not _NC_CACHE:
        _NC_CACHE.append(build_nc())
    nc = _NC_CACHE[0]
    in_maps = make_in_maps(inputs)
    res = run_bass_kernel_spmd(nc, in_maps, core_ids=list(range(NCORES)),
                               trace=trace, **kwargs)
    partials = np.stack([r["outp"] for r in res.results])      # [8, B, DIM]
    out = partials.sum(axis=0, dtype=np.float64).astype(np.float32)
    return out.reshape(B, 1, DIM), res


def kernel(**inputs):
    out, _ = run(inputs)
    return out


# revision 36
# speedup vs baseline: 1.6101x; 1.0715x over previous
"""GQA decode attention (B=32, S=1, 32 Q heads / 8 KV heads, HD=128, T=4096)
for 8 Trainium2 NeuronCores, tensor-parallel over heads.

Per core g: 4 query heads (4g..4g+3) + KV head g.
  - QKV projections from x (contraction over DIM=4096 on PE, fp32r)
  - RoPE on q/k (DVE, strided even/odd APs), 1/sqrt(HD) folded into q's rope
  - scores[bh, t] via zero-padded per-batch q weights (one PSUM bank per
    512-wide T chunk, accumulated over the 32 batches)
  - new-token k patched into the streamed K^T tile column 4095
  - softmax over the free axis on full 128 partitions (b*4+h), exp on ACT
    with fused accumulated row-sums; p normalized in place
  - p transposed per 128-chunk on PE; PV with V as stationary operand,
    output [d, bh] accumulated in one PSUM bank; new-token v added via one
    masked rank-32 matmul correction
  - out projection wo with attnT column-strided weights

Host side: shards weights/caches per head, pre-transposes K cache to
[B, HD, T] and pre-swizzles V cache to [B, 128, 32, HD] so every DMA is
16KB-contiguous per partition. Partial outputs (one per core) summed on host.
"""

import numpy as np

B, DIM, NH, NKV, HD = 32, 4096, 32, 8, 128
T = 4096
NCORES = 8
HPC = NH // NCORES            # 4 query heads per core
OUTW = HPC * HD               # 512
ALPHA = float(1.0 / np.sqrt(HD))
DC = DIM // 128               # 32 contraction chunks for projections
TC = T // 512                 # 8 score chunks (512 wide)
PC = T // 128                 # 32 PV chunks (128 deep)


KV_BF16 = True


def build_nc(dbg=False, corr=True, kv_bf16=KV_BF16):
    import concourse.mybir as mybir
    import concourse.tile as tile
    from concourse import bacc

    f32 = mybir.dt.float32
    f32r = mybir.dt.float32r
    kvdt = mybir.dt.float16 if kv_bf16 else f32
    X = mybir.AxisListType.X
    EXP = mybir.ActivationFunctionType.Exp
    SUB = mybir.AluOpType.subtract

    nc = bacc.Bacc("TRN2", target_bir_lowering=False, debug=False,
                   num_devices=NCORES)

    xT = nc.dram_tensor("xT", [DIM, B], f32, kind="ExternalInput")
    wq = nc.dram_tensor("wq", [DIM, OUTW], f32, kind="ExternalInput")
    wk = nc.dram_tensor("wk", [DIM, HD], f32, kind="ExternalInput")
    wv = nc.dram_tensor("wv", [DIM, HD], f32, kind="ExternalInput")
    wo = nc.dram_tensor("wo", [OUTW, DIM], f32, kind="ExternalInput")
    kt = nc.dram_tensor("kt", [B, HD, T], kvdt, kind="ExternalInput")
    vc = nc.dram_tensor("vc", [B, 128, PC, HD], kvdt, kind="ExternalInput")
    csq = nc.dram_tensor("csq", [2, OUTW // 2], f32, kind="ExternalInput")
    csk = nc.dram_tensor("csk", [2, HD // 2], f32, kind="ExternalInput")
    ones = nc.dram_tensor("ones", [1, 128], f32, kind="ExternalInput")
    iden = nc.dram_tensor("iden", [128, 128], f32, kind="ExternalInput")
    outp = nc.dram_tensor("outp", [B, DIM], f32, kind="ExternalOutput")
    if dbg:
        dbg_qrot = nc.dram_tensor("dbg_qrot", [B, OUTW], f32,
                                  kind="ExternalOutput")
        dbg_krot = nc.dram_tensor("dbg_krot", [B, HD], f32,
                                  kind="ExternalOutput")
        dbg_scores = nc.dram_tensor("dbg_scores", [128, T], f32,
                                    kind="ExternalOutput")
        dbg_p = nc.dram_tensor("dbg_p", [128, T], f32, kind="ExternalOutput")
        dbg_attnT = nc.dram_tensor("dbg_attnT", [128, B * HPC], f32,
                                   kind="ExternalOutput")
        dbg_prow = nc.dram_tensor("dbg_prow", [1, 128], f32,
                                  kind="ExternalOutput")

    def r(ap):
        return ap.bitcast(f32r)

    def rkv(ap):
        # KV-path matmul operand view: bf16 tiles pass through, f32 tiles
        # are viewed as f32r
        return ap if kv_bf16 else ap.bitcast(f32r)

    kv_bufs = 5 if kv_bf16 else 3

    with tile.TileContext(nc) as tc:
        with (
            tc.tile_pool(name="pp", bufs=1) as pp,
            tc.tile_pool(name="wp", bufs=4) as wp,
            tc.tile_pool(name="wkp", bufs=3) as wkp,
            tc.tile_pool(name="ktp", bufs=kv_bufs) as ktp,
            tc.tile_pool(name="vp", bufs=kv_bufs) as vp,
            tc.tile_pool(name="qxp", bufs=2) as qxp,
            tc.tile_pool(name="outp_pool", bufs=2) as outpp,
        ):
            # ------- constants & persistent tiles
            xT_sb = pp.tile([128, DC, B], f32, tag="xT_sb")
            nc.sync.dma_start(r(xT_sb),
                              r(xT[:].rearrange("(dc p) b -> p dc b", p=128)))
            iden_sb = pp.tile([128, 128], f32, tag="iden_sb")
            nc.sync.dma_start(iden_sb, iden[:])
            ones_sb = pp.tile([1, 128], f32, tag="ones_sb")
            nc.sync.dma_start(r(ones_sb), r(ones[:]))
            cq32 = pp.tile([B, OUTW // 2], f32, tag="cq32")
            nc.sync.dma_start(cq32, csq[0:1, :].to_broadcast([B, OUTW // 2]))
            sq32 = pp.tile([B, OUTW // 2], f32, tag="sq32")
            nc.sync.dma_start(sq32, csq[1:2, :].to_broadcast([B, OUTW // 2]))
            ck32 = pp.tile([B, HD // 2], f32, tag="ck32")
            nc.sync.dma_start(ck32, csk[0:1, :].to_broadcast([B, HD // 2]))
            sk32 = pp.tile([B, HD // 2], f32, tag="sk32")
            nc.sync.dma_start(sk32, csk[1:2, :].to_broadcast([B, HD // 2]))
            zero1 = pp.tile([128, 1], f32, tag="zero1")
            nc.vector.memset(zero1, 0.0)

            # ------- phase A: projections + rope + q/k transposes
            with tc.tile_pool(name="psA", bufs=1, space="PSUM") as psA:
                psq = psA.tile([B, OUTW], f32, tag="psq")
                for dc in range(DC):
                    wqt = wp.tile([128, OUTW], f32, tag="wchunk",
                                  name=f"wqt{dc}")
                    nc.sync.dma_start(
                        r(wqt),
                        r(wq[:].rearrange("(dc p) o -> dc p o", p=128)[dc]))
                    nc.tensor.matmul(psq, r(xT_sb[:, dc, :]), r(wqt),
                                     start=(dc == 0), stop=(dc == DC - 1))
                # fused K|V projection: one 32-matmul chain over [dc] with
                # a [128, 256] weight tile (wk cols then wv cols)
                pskv = psA.tile([B, 2 * HD], f32, tag="pskv")
                for dc in range(DC):
                    wkvt = wkp.tile([128, 2, HD], f32, tag="wkvt",
                                    name=f"wkvt{dc}")
                    nc.sync.dma_start(
                        r(wkvt[:, 0, :]),
                        r(wk[:].rearrange("(dc p) o -> dc p o", p=128)[dc]))
                    nc.sync.dma_start(
                        r(wkvt[:, 1, :]),
                        r(wv[:].rearrange("(dc p) o -> dc p o", p=128)[dc]))
                    nc.tensor.matmul(pskv, r(xT_sb[:, dc, :]),
                                     r(wkvt[:].rearrange("p a o -> p (a o)")),
                                     start=(dc == 0), stop=(dc == DC - 1))

                q_sb = pp.tile([B, OUTW], f32, tag="q_sb")
                nc.vector.tensor_copy(q_sb, psq)
                k_sb = pp.tile([B, HD], f32, tag="k_sb")
                nc.vector.tensor_copy(k_sb, pskv[:, 0:HD])
                vnew_sb = pp.tile([B, HD], f32, tag="vnew_sb")
                nc.vector.tensor_copy(vnew_sb, pskv[:, HD:2 * HD])

                # rope on q (scaled by alpha via csq) and k (unscaled)
                qrot = pp.tile([B, OUTW], f32, tag="qrot")
                tA = qxp.tile([B, OUTW // 2], f32, tag="ropetmp", name="tA")
                tB = qxp.tile([B, OUTW // 2], f32, tag="ropetmp", name="tB")
                qe, qo = q_sb[:, 0::2], q_sb[:, 1::2]
                nc.vector.tensor_mul(tA, qe, cq32)
                nc.vector.tensor_mul(tB, qo, sq32)
                nc.vector.tensor_tensor(qrot[:, 0::2], tA, tB, SUB)
                tC = qxp.tile([B, OUTW // 2], f32, tag="ropetmp", name="tC")
                tD = qxp.tile([B, OUTW // 2], f32, tag="ropetmp", name="tD")
                nc.vector.tensor_mul(tC, qe, sq32)
                nc.vector.tensor_mul(tD, qo, cq32)
                nc.vector.tensor_add(qrot[:, 1::2], tC, tD)

                krot = pp.tile([B, HD], f32, tag="krot")
                uA = qxp.tile([B, HD // 2], f32, tag="kropetmp", name="uA")
                uB = qxp.tile([B, HD // 2], f32, tag="kropetmp", name="uB")
                ke, ko = k_sb[:, 0::2], k_sb[:, 1::2]
                nc.vector.tensor_mul(uA, ke, ck32)
                nc.vector.tensor_mul(uB, ko, sk32)
                nc.vector.tensor_tensor(krot[:, 0::2], uA, uB, SUB)
                uC = qxp.tile([B, HD // 2], f32, tag="kropetmp", name="uC")
                uD = qxp.tile([B, HD // 2], f32, tag="kropetmp", name="uD")
                nc.vector.tensor_mul(uC, ke, sk32)
                nc.vector.tensor_mul(uD, ko, ck32)
                nc.vector.tensor_add(krot[:, 1::2], uC, uD)

                # transpose q (per head) and k_new to [d, b]
                qT_sb = pp.tile([128, HPC, B], f32, tag="qT_sb")
                for h in range(HPC):
                    pst = psA.tile([128, B], f32, tag="pstA",
                                   name=f"pstA{h}")
                    nc.tensor.transpose(pst, qrot[:, h * HD:(h + 1) * HD],
                                        iden_sb[0:B, 0:B])
                    nc.vector.tensor_copy(qT_sb[:, h, :], pst)
                ktnew_sb = pp.tile([128, B], kvdt, tag="ktnew_sb")
                pstk = psA.tile([128, B], f32, tag="pstA")
                nc.tensor.transpose(pstk, krot, iden_sb[0:B, 0:B])
                nc.vector.tensor_copy(rkv(ktnew_sb), pstk)
                vnewT_sb = pp.tile([128, B], f32, tag="vnewT_sb")
                pstv = psA.tile([128, B], f32, tag="pstA")
                nc.tensor.transpose(pstv, vnew_sb, iden_sb[0:B, 0:B])
                nc.vector.tensor_copy(vnewT_sb, pstv)

                # zero-padded per-batch q weights [d, bh], batch b in block b
                qxall = pp.tile([128, B * 128], kvdt, tag="qxall")
                nc.vector.tensor_copy(
                    rkv(qxall), zero1[:, 0:1].to_broadcast([128, B * 128]))
                for b in range(B):
                    nc.vector.tensor_copy(
                        rkv(qxall[:, 128 * b + HPC * b:128 * b
                                  + HPC * (b + 1)]),
                        qT_sb[:, :, b])

            # ------- phase B: QK scores
            scores = pp.tile([128, T], f32, tag="scores")
            with tc.tile_pool(name="psB", bufs=1, space="PSUM") as psB:
                pqk = [psB.tile([128, 512], f32, tag=f"pqk{c}",
                                name=f"pqk{c}")
                       for c in range(TC)]
                for b in range(B):
                    ktb = ktp.tile([128, T], kvdt, tag="ktb", name=f"ktb{b}")
                    for q4 in range(4):
                        nc.sync.dma_start(
                            rkv(ktb[:, q4 * 1024:(q4 + 1) * 1024]),
                            rkv(kt[b, :, q4 * 1024:(q4 + 1) * 1024]))
                    # patch the new-token k into cache column 4095
                    nc.vector.tensor_copy(rkv(ktb[:, T - 1:T]),
                                          ktnew_sb[:, b:b + 1])
                    for c in range(TC):
                        nc.tensor.matmul(
                            pqk[c],
                            rkv(qxall[:, 128 * b:128 * (b + 1)]),
                            rkv(ktb[:, c * 512:(c + 1) * 512]),
                            start=(b == 0), stop=(b == B - 1))
                for c in range(TC):
                    nc.vector.tensor_copy(scores[:, c * 512:(c + 1) * 512],
                                          pqk[c])
            if dbg:
                nc.sync.dma_start(dbg_qrot[:], qrot)
                nc.sync.dma_start(dbg_krot[:], krot)
                nc.sync.dma_start(dbg_scores[:], scores)

            # ------- softmax (rows = b*4+h on partitions)
            maxv = pp.tile([128, 1], f32, tag="maxv")
            nc.vector.reduce_max(maxv, scores, axis=X)
            negmax = pp.tile([128, 1], f32, tag="negmax")
            nc.vector.tensor_scalar_mul(negmax, maxv, -1.0)
            sums = pp.tile([128, 1], f32, tag="sums")
            nc.scalar.activation(scores, scores, EXP, bias=negmax, scale=1.0,
                                 accum_out=sums)
            recip = pp.tile([128, 1], f32, tag="recip")
            nc.vector.reciprocal(recip, sums)
            nc.vector.tensor_scalar_mul(scores, scores, recip)

            # ------- phase C: transpose p, PV, out projection
            with tc.tile_pool(name="psC", bufs=2, space="PSUM") as psC:
                # extract p[:, 4095] (new-token weights), then zero that
                # column so the stale cache row at t=4095 contributes nothing;
                # the real new-token v is added via the masked correction
                # matmul below
                psr = psC.tile([1, 128], f32, tag="psr", bufs=1)
                nc.tensor.transpose(psr, scores[:, T - 1:T], iden_sb)
                prow = pp.tile([1, 128], f32, tag="prow")
                nc.vector.tensor_copy(r(prow), psr)
                nc.vector.tensor_copy(scores[:, T - 1:T], zero1)
                if dbg:
                    nc.sync.dma_start(dbg_p[:], scores)
                # broadcast prow to all 128 partitions via rank-1 outer product
                psbc = psC.tile([128, 128], f32, tag="psbc", bufs=1)
                nc.tensor.matmul(psbc, r(ones_sb), r(prow))

                pT = pp.tile([128, PC, 128], kvdt, tag="pT")
                for c2 in range(PC):
                    pstx = psC.tile([128, 128], f32, tag="pstx",
                                    name=f"pstx{c2}")
                    nc.tensor.transpose(pstx,
                                        scores[:, c2 * 128:(c2 + 1) * 128],
                                        iden_sb)
                    nc.vector.tensor_copy(rkv(pT[:, c2, :]), pstx)
                if dbg:
                    nc.sync.dma_start(dbg_prow[:], prow)

                psat = psC.tile([128, B * HPC], f32, tag="psat", bufs=1)
                for b in range(B):
                    vb = vp.tile([128, PC, HD], kvdt, tag="vb", name=f"vb{b}")
                    for q4 in range(4):
                        nc.sync.dma_start(rkv(vb[:, q4 * 8:(q4 + 1) * 8, :]),
                                          rkv(vc[b, :, q4 * 8:(q4 + 1) * 8, :]))
                    for c2 in range(PC):
                        nc.tensor.matmul(
                            psat[:, HPC * b:HPC * (b + 1)],
                            rkv(vb[:, c2, :]),
                            rkv(pT[:, c2, HPC * b:HPC * (b + 1)]),
                            start=(c2 == 0), stop=(c2 == PC - 1),
                            skip_group_check=True)

                # new-token correction on DVE:
                # corrT[d, 4b+h] = vnewT[d, b] * prow[4b+h]
                corrT = pp.tile([128, B, HPC], f32, tag="corrT")
                nc.vector.tensor_mul(
                    corrT,
                    vnewT_sb[:, :, None].to_broadcast([128, B, HPC]),
                    psbc[:].rearrange("d (b h) -> d b h", h=HPC))
                attnT = pp.tile([128, B * HPC], f32, tag="attnT")
                if corr:
                    nc.vector.tensor_add(
                        r(attnT), psat,
                        corrT[:].rearrange("d b h -> d (b h)"))
                else:
                    nc.vector.tensor_copy(r(attnT), psat)
                if dbg:
                    nc.sync.dma_start(dbg_attnT[:], attnT)

                for ncc in range(8):
                    pso = psC.tile([B, 512], f32, tag="pso", name=f"pso{ncc}")
                    for h in range(HPC):
                        wot = wp.tile([128, 512], f32, tag="wchunk",
                                      name=f"wot{ncc}_{h}")
                        nc.sync.dma_start(
                            r(wot), r(wo[h * HD:(h + 1) * HD,
                                         ncc * 512:(ncc + 1) * 512]))
                        nc.tensor.matmul(pso, r(attnT[:, h::HPC]), r(wot),
                                         start=(h == 0), stop=(h == HPC - 1))
                    osb = outpp.tile([B, 512], f32, tag="osb",
                                     name=f"osb{ncc}")
                    nc.vector.tensor_copy(osb, pso)
                    nc.sync.dma_start(outp[:, ncc * 512:(ncc + 1) * 512], osb)

    nc.compile()
    return nc


def make_in_maps(inputs):
    x = np.asarray(inputs["x"], np.float32).reshape(B, DIM)
    cache_k = np.asarray(inputs["cache_k"], np.float32)
    cache_v = np.asarray(inputs["cache_v"], np.float32)
    wq = np.asarray(inputs["wq"], np.float32)
    wk = np.asarray(inputs["wk"], np.float32)
    wv = np.asarray(inputs["wv"], np.float32)
    wo = np.asarray(inputs["wo"], np.float32)
    cos = np.asarray(inputs["freqs_cos"], np.float32).reshape(-1)
    sin = np.asarray(inputs["freqs_sin"], np.float32).reshape(-1)

    xT = np.ascontiguousarray(x.T)                             # [DIM, B]
    csq = np.ascontiguousarray(
        np.stack([np.tile(cos, HPC), np.tile(sin, HPC)]) * ALPHA)
    csk = np.ascontiguousarray(np.stack([cos, sin]))
    onesv = np.ones((1, 128), np.float32)
    iden = np.eye(128, dtype=np.float32)

    kv_np = np.float16 if KV_BF16 else np.float32

    in_maps = []
    for g in range(NCORES):
        kt_g = np.ascontiguousarray(
            cache_k[:, :, g, :].transpose(0, 2, 1)).astype(kv_np)  # [B,HD,T]
        v_g = np.ascontiguousarray(
            cache_v[:, :, g, :].reshape(B, PC, 128, HD)
            .transpose(0, 2, 1, 3)).astype(kv_np)              # [B,128,PC,HD]
        in_maps.append({
            "xT": xT,
            "wq": np.ascontiguousarray(wq[:, g * OUTW:(g + 1) * OUTW]),
            "wk": np.ascontiguousarray(wk[:, g * HD:(g + 1) * HD]),
            "wv": np.ascontiguousarray(wv[:, g * HD:(g + 1) * HD]),
            "wo": np.ascontiguousarray(wo[g * OUTW:(g + 1) * OUTW, :]),
            "kt": kt_g,
            "vc": v_g,
            "csq": csq,
            "csk": csk,
            "ones": onesv,
            "iden": iden,
        })
    return in_maps


_NC_CACHE = []


def run(inputs, trace=False, **kwargs):
    from concourse.bass_utils import run_bass_kernel_spmd
    if not _NC_CACHE:
        _NC_CACHE.append(build_nc())
    nc = _NC_CACHE[0]
    in_maps = make_in_maps(inputs)
    res = run_bass_kernel_spmd(nc, in_maps, core_ids=list(range(NCORES)),
                               trace=trace, **kwargs)
    partials = np.stack([r["outp"] for r in res.results])      # [8, B, DIM]
    out = partials.sum(axis=0, dtype=np.float64).astype(np.float32)
    return out.reshape(B, 1, DIM), res


def kernel(**inputs):
    out, _ = run(inputs)
    return out


# revision 39
# speedup vs baseline: 1.7155x; 1.0655x over previous
"""GQA decode attention (B=32, S=1, 32 Q heads / 8 KV heads, HD=128, T=4096)
for 8 Trainium2 NeuronCores, tensor-parallel over heads.

Per core g: 4 query heads (4g..4g+3) + KV head g.
  - QKV projections from x (contraction over DIM=4096 on PE, fp32r)
  - RoPE on q/k (DVE, strided even/odd APs), 1/sqrt(HD) folded into q's rope
  - scores[bh, t] via zero-padded per-batch q weights (one PSUM bank per
    512-wide T chunk, accumulated over the 32 batches)
  - new-token k patched into the streamed K^T tile column 4095
  - softmax over the free axis on full 128 partitions (b*4+h), exp on ACT
    with fused accumulated row-sums; p normalized in place
  - p transposed per 128-chunk on PE; PV with V as stationary operand,
    output [d, bh] accumulated in one PSUM bank; new-token v added via one
    masked rank-32 matmul correction
  - out projection wo with attnT column-strided weights

Host side: shards weights/caches per head, pre-transposes K cache to
[B, HD, T] and pre-swizzles V cache to [B, 128, 32, HD] so every DMA is
16KB-contiguous per partition. Partial outputs (one per core) summed on host.
"""

import numpy as np

B, DIM, NH, NKV, HD = 32, 4096, 32, 8, 128
T = 4096
NCORES = 8
HPC = NH // NCORES            # 4 query heads per core
OUTW = HPC * HD               # 512
ALPHA = float(1.0 / np.sqrt(HD))
DC = DIM // 128               # 32 contraction chunks for projections
TC = T // 512                 # 8 score chunks (512 wide)
PC = T // 128                 # 32 PV chunks (128 deep)


KV_BF16 = True


def build_nc(dbg=False, corr=True, kv_bf16=KV_BF16):
    import concourse.mybir as mybir
    import concourse.tile as tile
    from concourse import bacc

    f32 = mybir.dt.float32
    f32r = mybir.dt.float32r
    kvdt = mybir.dt.float16 if kv_bf16 else f32
    X = mybir.AxisListType.X
    EXP = mybir.ActivationFunctionType.Exp
    SUB = mybir.AluOpType.subtract

    nc = bacc.Bacc("TRN2", target_bir_lowering=False, debug=False,
                   num_devices=NCORES)

    xT = nc.dram_tensor("xT", [DIM, B], kvdt, kind="ExternalInput")
    wq = nc.dram_tensor("wq", [DIM, OUTW], kvdt, kind="ExternalInput")
    wk = nc.dram_tensor("wk", [DIM, HD], kvdt, kind="ExternalInput")
    wv = nc.dram_tensor("wv", [DIM, HD], kvdt, kind="ExternalInput")
    wo = nc.dram_tensor("wo", [OUTW, DIM], kvdt, kind="ExternalInput")
    kt = nc.dram_tensor("kt", [B, HD, T], kvdt, kind="ExternalInput")
    vc = nc.dram_tensor("vc", [B, 128, PC, HD], kvdt, kind="ExternalInput")
    csq = nc.dram_tensor("csq", [2, OUTW // 2], f32, kind="ExternalInput")
    csk = nc.dram_tensor("csk", [2, HD // 2], f32, kind="ExternalInput")
    ones = nc.dram_tensor("ones", [1, 128], f32, kind="ExternalInput")
    iden = nc.dram_tensor("iden", [128, 128], f32, kind="ExternalInput")
    outp = nc.dram_tensor("outp", [B, DIM], f32, kind="ExternalOutput")
    if dbg:
        dbg_qrot = nc.dram_tensor("dbg_qrot", [B, OUTW], f32,
                                  kind="ExternalOutput")
        dbg_krot = nc.dram_tensor("dbg_krot", [B, HD], f32,
                                  kind="ExternalOutput")
        dbg_scores = nc.dram_tensor("dbg_scores", [128, T], f32,
                                    kind="ExternalOutput")
        dbg_p = nc.dram_tensor("dbg_p", [128, T], f32, kind="ExternalOutput")
        dbg_attnT = nc.dram_tensor("dbg_attnT", [128, B * HPC], f32,
                                   kind="ExternalOutput")
        dbg_prow = nc.dram_tensor("dbg_prow", [1, 128], f32,
                                  kind="ExternalOutput")

    def r(ap):
        return ap.bitcast(f32r)

    def rkv(ap):
        # KV-path matmul operand view: bf16 tiles pass through, f32 tiles
        # are viewed as f32r
        return ap if kv_bf16 else ap.bitcast(f32r)

    kv_bufs = 6 if kv_bf16 else 3

    with tile.TileContext(nc) as tc:
        with (
            tc.tile_pool(name="pp", bufs=1) as pp,
            tc.tile_pool(name="wp", bufs=8) as wp,
            tc.tile_pool(name="wkp", bufs=8) as wkp,
            tc.tile_pool(name="ktp", bufs=kv_bufs) as ktp,
            tc.tile_pool(name="vp", bufs=kv_bufs) as vp,
            tc.tile_pool(name="qxp", bufs=2) as qxp,
            tc.tile_pool(name="outp_pool", bufs=2) as outpp,
        ):
            # ------- constants & persistent tiles
            xT_sb = pp.tile([128, DC, B], kvdt, tag="xT_sb")
            nc.sync.dma_start(rkv(xT_sb),
                              rkv(xT[:].rearrange("(dc p) b -> p dc b",
                                                  p=128)))
            iden_sb = pp.tile([128, 128], f32, tag="iden_sb")
            nc.sync.dma_start(iden_sb, iden[:])
            ones_sb = pp.tile([1, 128], f32, tag="ones_sb")
            nc.sync.dma_start(r(ones_sb), r(ones[:]))
            cq32 = pp.tile([B, OUTW // 2], f32, tag="cq32")
            nc.sync.dma_start(cq32, csq[0:1, :].to_broadcast([B, OUTW // 2]))
            sq32 = pp.tile([B, OUTW // 2], f32, tag="sq32")
            nc.sync.dma_start(sq32, csq[1:2, :].to_broadcast([B, OUTW // 2]))
            ck32 = pp.tile([B, HD // 2], f32, tag="ck32")
            nc.sync.dma_start(ck32, csk[0:1, :].to_broadcast([B, HD // 2]))
            sk32 = pp.tile([B, HD // 2], f32, tag="sk32")
            nc.sync.dma_start(sk32, csk[1:2, :].to_broadcast([B, HD // 2]))
            zero1 = pp.tile([128, 1], f32, tag="zero1")
            nc.vector.memset(zero1, 0.0)

            # ------- phase A: projections + rope + q/k transposes
            with tc.tile_pool(name="psA", bufs=1, space="PSUM") as psA:
                psq = psA.tile([B, OUTW], f32, tag="psq")
                for dc in range(DC):
                    wqt = wp.tile([128, OUTW], kvdt, tag="wchunk",
                                  name=f"wqt{dc}")
                    nc.sync.dma_start(
                        rkv(wqt),
                        rkv(wq[:].rearrange("(dc p) o -> dc p o", p=128)[dc]))
                    nc.tensor.matmul(psq, rkv(xT_sb[:, dc, :]), rkv(wqt),
                                     start=(dc == 0), stop=(dc == DC - 1))
                # fused K|V projection: one 32-matmul chain over [dc] with
                # a [128, 256] weight tile (wk cols then wv cols)
                pskv = psA.tile([B, 2 * HD], f32, tag="pskv")
                for dc in range(DC):
                    wkvt = wkp.tile([128, 2, HD], kvdt, tag="wkvt",
                                    name=f"wkvt{dc}")
                    nc.sync.dma_start(
                        rkv(wkvt[:, 0, :]),
                        rkv(wk[:].rearrange("(dc p) o -> dc p o", p=128)[dc]))
                    nc.sync.dma_start(
                        rkv(wkvt[:, 1, :]),
                        rkv(wv[:].rearrange("(dc p) o -> dc p o", p=128)[dc]))
                    nc.tensor.matmul(pskv, rkv(xT_sb[:, dc, :]),
                                     rkv(wkvt[:].rearrange("p a o -> p (a o)")),
                                     start=(dc == 0), stop=(dc == DC - 1))

                q_sb = pp.tile([B, OUTW], f32, tag="q_sb")
                nc.vector.tensor_copy(q_sb, psq)
                k_sb = pp.tile([B, HD], f32, tag="k_sb")
                nc.vector.tensor_copy(k_sb, pskv[:, 0:HD])
                vnew_sb = pp.tile([B, HD], f32, tag="vnew_sb")
                nc.vector.tensor_copy(vnew_sb, pskv[:, HD:2 * HD])

                # rope on q (scaled by alpha via csq) and k (unscaled)
                qrot = pp.tile([B, OUTW], f32, tag="qrot")
                tA = qxp.tile([B, OUTW // 2], f32, tag="ropetmp", name="tA")
                tB = qxp.tile([B, OUTW // 2], f32, tag="ropetmp", name="tB")
                qe, qo = q_sb[:, 0::2], q_sb[:, 1::2]
                nc.vector.tensor_mul(tA, qe, cq32)
                nc.vector.tensor_mul(tB, qo, sq32)
                nc.vector.tensor_tensor(qrot[:, 0::2], tA, tB, SUB)
                tC = qxp.tile([B, OUTW // 2], f32, tag="ropetmp", name="tC")
                tD = qxp.tile([B, OUTW // 2], f32, tag="ropetmp", name="tD")
                nc.vector.tensor_mul(tC, qe, sq32)
                nc.vector.tensor_mul(tD, qo, cq32)
                nc.vector.tensor_add(qrot[:, 1::2], tC, tD)

                krot = pp.tile([B, HD], f32, tag="krot")
                uA = qxp.tile([B, HD // 2], f32, tag="kropetmp", name="uA")
                uB = qxp.tile([B, HD // 2], f32, tag="kropetmp", name="uB")
                ke, ko = k_sb[:, 0::2], k_sb[:, 1::2]
                nc.vector.tensor_mul(uA, ke, ck32)
                nc.vector.tensor_mul(uB, ko, sk32)
                nc.vector.tensor_tensor(krot[:, 0::2], uA, uB, SUB)
                uC = qxp.tile([B, HD // 2], f32, tag="kropetmp", name="uC")
                uD = qxp.tile([B, HD // 2], f32, tag="kropetmp", name="uD")
                nc.vector.tensor_mul(uC, ke, sk32)
                nc.vector.tensor_mul(uD, ko, ck32)
                nc.vector.tensor_add(krot[:, 1::2], uC, uD)

                # transpose q (per head) and k_new to [d, b]
                qT_sb = pp.tile([128, HPC, B], f32, tag="qT_sb")
                for h in range(HPC):
                    pst = psA.tile([128, B], f32, tag="pstA",
                                   name=f"pstA{h}")
                    nc.tensor.transpose(pst, qrot[:, h * HD:(h + 1) * HD],
                                        iden_sb[0:B, 0:B])
                    nc.vector.tensor_copy(qT_sb[:, h, :], pst)
                ktnew_sb = pp.tile([128, B], kvdt, tag="ktnew_sb")
                pstk = psA.tile([128, B], f32, tag="pstA")
                nc.tensor.transpose(pstk, krot, iden_sb[0:B, 0:B])
                nc.vector.tensor_copy(rkv(ktnew_sb), pstk)
                vnewT_sb = pp.tile([128, B], f32, tag="vnewT_sb")
                pstv = psA.tile([128, B], f32, tag="pstA")
                nc.tensor.transpose(pstv, vnew_sb, iden_sb[0:B, 0:B])
                nc.vector.tensor_copy(vnewT_sb, pstv)

                # zero-padded per-batch q weights [d, bh], batch b in block b
                qxall = pp.tile([128, B * 128], kvdt, tag="qxall")
                nc.vector.tensor_copy(
                    rkv(qxall), zero1[:, 0:1].to_broadcast([128, B * 128]))
                for b in range(B):
                    nc.vector.tensor_copy(
                        rkv(qxall[:, 128 * b + HPC * b:128 * b
                                  + HPC * (b + 1)]),
                        qT_sb[:, :, b])

            # ------- phase B: QK scores
            scores = pp.tile([128, T], f32, tag="scores")
            with tc.tile_pool(name="psB", bufs=1, space="PSUM") as psB:
                pqk = [psB.tile([128, 512], f32, tag=f"pqk{c}",
                                name=f"pqk{c}")
                       for c in range(TC)]
                for b in range(B):
                    ktb = ktp.tile([128, T], kvdt, tag="ktb", name=f"ktb{b}")
                    for q4 in range(4):
                        nc.sync.dma_start(
                            rkv(ktb[:, q4 * 1024:(q4 + 1) * 1024]),
                            rkv(kt[b, :, q4 * 1024:(q4 + 1) * 1024]))
                    # patch the new-token k into cache column 4095
                    nc.vector.tensor_copy(rkv(ktb[:, T - 1:T]),
                                          ktnew_sb[:, b:b + 1])
                    for c in range(TC):
                        nc.tensor.matmul(
                            pqk[c],
                            rkv(qxall[:, 128 * b:128 * (b + 1)]),
                            rkv(ktb[:, c * 512:(c + 1) * 512]),
                            start=(b == 0), stop=(b == B - 1))
                for c in range(TC):
                    nc.vector.tensor_copy(scores[:, c * 512:(c + 1) * 512],
                                          pqk[c])
            if dbg:
                nc.sync.dma_start(dbg_qrot[:], qrot)
                nc.sync.dma_start(dbg_krot[:], krot)
                nc.sync.dma_start(dbg_scores[:], scores)

            # ------- softmax (rows = b*4+h on partitions)
            maxv = pp.tile([128, 1], f32, tag="maxv")
            nc.vector.reduce_max(maxv, scores, axis=X)
            negmax = pp.tile([128, 1], f32, tag="negmax")
            nc.vector.tensor_scalar_mul(negmax, maxv, -1.0)
            sums = pp.tile([128, 1], f32, tag="sums")
            nc.scalar.activation(scores, scores, EXP, bias=negmax, scale=1.0,
                                 accum_out=sums)
            recip = pp.tile([128, 1], f32, tag="recip")
            nc.vector.reciprocal(recip, sums)
            nc.vector.tensor_scalar_mul(scores, scores, recip)

            # ------- phase C: transpose p, PV, out projection
            with tc.tile_pool(name="psC", bufs=2, space="PSUM") as psC:
                # extract p[:, 4095] (new-token weights), then zero that
                # column so the stale cache row at t=4095 contributes nothing;
                # the real new-token v is added via the masked correction
                # matmul below
                psr = psC.tile([1, 128], f32, tag="psr", bufs=1)
                nc.tensor.transpose(psr, scores[:, T - 1:T], iden_sb)
                prow = pp.tile([1, 128], f32, tag="prow")
                nc.vector.tensor_copy(r(prow), psr)
                nc.vector.tensor_copy(scores[:, T - 1:T], zero1)
                if dbg:
                    nc.sync.dma_start(dbg_p[:], scores)
                # broadcast prow to all 128 partitions via rank-1 outer product
                psbc = psC.tile([128, 128], f32, tag="psbc", bufs=1)
                nc.tensor.matmul(psbc, r(ones_sb), r(prow))

                pT = pp.tile([128, PC, 128], kvdt, tag="pT")
                for c2 in range(PC):
                    pstx = psC.tile([128, 128], f32, tag="pstx",
                                    name=f"pstx{c2}")
                    nc.tensor.transpose(pstx,
                                        scores[:, c2 * 128:(c2 + 1) * 128],
                                        iden_sb)
                    nc.vector.tensor_copy(rkv(pT[:, c2, :]), pstx)
                if dbg:
                    nc.sync.dma_start(dbg_prow[:], prow)

                psat = psC.tile([128, B * HPC], f32, tag="psat", bufs=1)
                for b in range(B):
                    vb = vp.tile([128, PC, HD], kvdt, tag="vb", name=f"vb{b}")
                    for q4 in range(4):
                        nc.sync.dma_start(rkv(vb[:, q4 * 8:(q4 + 1) * 8, :]),
                                          rkv(vc[b, :, q4 * 8:(q4 + 1) * 8, :]))
                    for c2 in range(PC):
                        nc.tensor.matmul(
                            psat[:, HPC * b:HPC * (b + 1)],
                            rkv(vb[:, c2, :]),
                            rkv(pT[:, c2, HPC * b:HPC * (b + 1)]),
                            start=(c2 == 0), stop=(c2 == PC - 1),
                            skip_group_check=True)

                # new-token correction on DVE:
                # corrT[d, 4b+h] = vnewT[d, b] * prow[4b+h]
                corrT = pp.tile([128, B, HPC], f32, tag="corrT")
                nc.vector.tensor_mul(
                    corrT,
                    vnewT_sb[:, :, None].to_broadcast([128, B, HPC]),
                    psbc[:].rearrange("d (b h) -> d b h", h=HPC))
                attnT = pp.tile([128, B * HPC], kvdt, tag="attnT")
                if corr:
                    nc.vector.tensor_add(
                        rkv(attnT), psat,
                        corrT[:].rearrange("d b h -> d (b h)"))
                else:
                    nc.vector.tensor_copy(rkv(attnT), psat)
                if dbg:
                    nc.sync.dma_start(dbg_attnT[:], attnT)

                for ncc in range(8):
                    pso = psC.tile([B, 512], f32, tag="pso", name=f"pso{ncc}")
                    for h in range(HPC):
                        wot = wp.tile([128, 512], kvdt, tag="wchunk",
                                      name=f"wot{ncc}_{h}")
                        nc.sync.dma_start(
                            rkv(wot), rkv(wo[h * HD:(h + 1) * HD,
                                             ncc * 512:(ncc + 1) * 512]))
                        nc.tensor.matmul(pso, rkv(attnT[:, h::HPC]), rkv(wot),
                                         start=(h == 0), stop=(h == HPC - 1))
                    osb = outpp.tile([B, 512], f32, tag="osb",
                                     name=f"osb{ncc}")
                    nc.vector.tensor_copy(osb, pso)
                    nc.sync.dma_start(outp[:, ncc * 512:(ncc + 1) * 512], osb)

    nc.compile()
    return nc


def make_in_maps(inputs):
    x = np.asarray(inputs["x"], np.float32).reshape(B, DIM)
    cache_k = np.asarray(inputs["cache_k"], np.float32)
    cache_v = np.asarray(inputs["cache_v"], np.float32)
    wq = np.asarray(inputs["wq"], np.float32)
    wk = np.asarray(inputs["wk"], np.float32)
    wv = np.asarray(inputs["wv"], np.float32)
    wo = np.asarray(inputs["wo"], np.float32)
    cos = np.asarray(inputs["freqs_cos"], np.float32).reshape(-1)
    sin = np.asarray(inputs["freqs_sin"], np.float32).reshape(-1)

    kv_np = np.float16 if KV_BF16 else np.float32
    xT = np.ascontiguousarray(x.T).astype(kv_np)               # [DIM, B]
    csq = np.ascontiguousarray(
        np.stack([np.tile(cos, HPC), np.tile(sin, HPC)]) * ALPHA)
    csk = np.ascontiguousarray(np.stack([cos, sin]))
    onesv = np.ones((1, 128), np.float32)
    iden = np.eye(128, dtype=np.float32)

    in_maps = []
    for g in range(NCORES):
        kt_g = np.ascontiguousarray(
            cache_k[:, :, g, :].transpose(0, 2, 1)).astype(kv_np)  # [B,HD,T]
        v_g = np.ascontiguousarray(
            cache_v[:, :, g, :].reshape(B, PC, 128, HD)
            .transpose(0, 2, 1, 3)).astype(kv_np)              # [B,128,PC,HD]
        in_maps.append({
            "xT": xT,
            "wq": np.ascontiguousarray(
                wq[:, g * OUTW:(g + 1) * OUTW]).astype(kv_np),
            "wk": np.ascontiguousarray(
                wk[:, g * HD:(g + 1) * HD]).astype(kv_np),
            "wv": np.ascontiguousarray(
                wv[:, g * HD:(g + 1) * HD]).astype(kv_np),
            "wo": np.ascontiguousarray(
                wo[g * OUTW:(g + 1) * OUTW, :]).astype(kv_np),
            "kt": kt_g,
            "vc": v_g,
            "csq": csq,
            "csk": csk,
            "ones": onesv,
            "iden": iden,
        })
    return in_maps


_NC_CACHE = []


def run(inputs, trace=False, **kwargs):
    from concourse.bass_utils import run_bass_kernel_spmd
    if not _NC_CACHE:
        _NC_CACHE.append(build_nc())
    nc = _NC_CACHE[0]
    in_maps = make_in_maps(inputs)
    res = run_bass_kernel_spmd(nc, in_maps, core_ids=list(range(NCORES)),
                               trace=trace, **kwargs)
    partials = np.stack([r["outp"] for r in res.results])      # [8, B, DIM]
    out = partials.sum(axis=0, dtype=np.float64).astype(np.float32)
    return out.reshape(B, 1, DIM), res


def kernel(**inputs):
    out, _ = run(inputs)
    return out
